# revision 17
# baseline (speedup 1.0000x reference)
"""Trainium2 Bass kernel for BFS Neural Execution (GNN message passing).

Math (reference):
    z   = relu([x, pre_h] @ enc_w + enc_b)                         [N,H]
    m_e = relu([z[tgt_e], z[src_e], attr_e] @ M_w + M_b)           [E,H]
    agg = segment_max(m, tgt) (empty -> 0)                         [N,H]
    h   = relu([z, agg] @ U_w + U_b)                               [N,H]
    y   = sigmoid([z, h] @ dec_w + dec_b)                          [N,1]
    ter = mean([h, mean(h)] @ ter_w + ter_b)                       scalar

Refactoring used here: with A = z@M_w[:H]+M_b, B = z@M_w[H:2H], w_a = M_w[2H],
    m_e = relu(A[tgt] + B[src] + attr_e*w_a)
and since relu is monotone and A[tgt] is constant within a segment,
    agg[n] = relu(A[n] + max_{e->n}(B[src_e] + attr_e*w_a))
with empty segments handled by a -1e30 sentinel (relu(-huge) == 0).

Distribution: edges are sharded by contiguous target-node ranges into 16
sub-shards (2 per NeuronCore, processed sequentially).  Each sub-shard
relabels the source nodes it references (<=~27k distinct, fits int16) and the
device builds a bf16 B-table for exactly those nodes, then uses the custom
dma_gather (transpose mode) to fetch B rows feature-major, adds attr*w_a, and
does a segmented max via strided reduce_max over degree-padded slot groups.
The slot schedule (per-position padded degree) is shared across all 16
sub-shards so a single NEFF runs SPMD on all 8 cores.
"""

import math
import os
import sys

import numpy as np

for _p in ("/opt/trn_rl_repo", "/opt/trn_rl_repo/concourse"):
    if _p not in sys.path:
        sys.path.insert(0, _p)

import ml_dtypes

P = 128
H = 128
NCORES = 8
NSUB = 2              # sub-shards per core
NSHARD = NCORES * NSUB
CCHUNK = 8192         # max gather-chunk columns (edge slots) per dma_gather
NEG = -1.0e30
# Allowed padded degrees (slot sizes).  Must be sorted ascending.
D_BUCKETS = [1, 2, 3, 4, 5, 6, 7, 8, 10, 12, 14, 16, 20, 24, 28, 32, 40, 48,
             64, 80, 96, 128, 192, 256, 384, 512, 768, 1024, 2048, 4096]


def _round_up(v, m):
    return -(-v // m) * m


def _bucket(d):
    if d == 0:
        return 0
    for b in D_BUCKETS:
        if b >= d:
            return b
    raise ValueError(f"degree {d} exceeds max bucket")


# ----------------------------------------------------------------------------
# Host-side planning
# ----------------------------------------------------------------------------

class Plan:
    pass


def build_plan(edge_index, n_nodes):
    """Shard edges by contiguous tgt ranges; build the shared slot schedule and
    per-shard index/relabel data."""
    src = np.asarray(edge_index[0], np.int64)
    tgt = np.asarray(edge_index[1], np.int64)
    E = src.shape[0]

    deg_all = np.bincount(tgt, minlength=n_nodes)
    cum = np.cumsum(deg_all)
    bounds = [0]
    for i in range(1, NSHARD):
        bounds.append(int(np.searchsorted(cum, E * i / NSHARD)))
    bounds.append(n_nodes)

    order = np.argsort(tgt, kind="stable")
    src_s, tgt_s = src[order], tgt[order]
    attr_perm = order  # edge permutation; attr gathered later

    # cut points in the sorted edge array per shard
    eb = [0]
    for i in range(1, NSHARD):
        eb.append(int(np.searchsorted(tgt_s, bounds[i])))
    eb.append(E)

    shards = []
    for s in range(NSHARD):
        sh = Plan()
        sh.n0, sh.n1 = bounds[s], bounds[s + 1]
        sh.e0, sh.e1 = eb[s], eb[s + 1]
        sh.src = src_s[sh.e0:sh.e1]
        sh.tgt = tgt_s[sh.e0:sh.e1]
        sh.attr_eidx = attr_perm[sh.e0:sh.e1]  # original edge ids
        sh.n_own = sh.n1 - sh.n0
        sh.deg = np.bincount(sh.tgt - sh.n0, minlength=sh.n_own)
        # node order: sorted by degree desc (stable)
        sh.node_order = np.argsort(-sh.deg, kind="stable")  # local ids
        sh.sorted_deg = sh.deg[sh.node_order]
        # distinct referenced srcs, ascending; relabel map
        sh.ref = np.unique(sh.src)
        shards.append(sh)

    pl = Plan()
    pl.n_nodes = n_nodes
    pl.E = E
    pl.shards = shards
    pl.S_OWN = _round_up(max(sh.n_own for sh in shards), 512)
    pl.T_TAB = _round_up(max(len(sh.ref) for sh in shards) + 1, 512)
    assert pl.T_TAB - 1 <= 32767, f"table too large for int16 idx: {pl.T_TAB}"
    pl.DUMTOK = pl.T_TAB - 1

    # shared slot schedule: profile[i] = max over shards of sorted_deg[i]
    prof = np.zeros(pl.S_OWN, np.int64)
    for sh in shards:
        prof[: sh.n_own] = np.maximum(prof[: sh.n_own], sh.sorted_deg)
    slot_d = np.array([_bucket(int(d)) for d in prof], np.int64)
    pl.slot_d = slot_d
    pl.n_real_slots = int((slot_d > 0).sum())  # trailing slots are d==0

    # chunking: group consecutive slots, <= CCHUNK padded columns per chunk
    chunks = []
    i = 0
    while i < pl.n_real_slots:
        cols = 0
        j = i
        runs = []  # [col_off, node_off, n_nodes, d]
        while j < pl.n_real_slots:
            d = int(slot_d[j])
            if cols + d > CCHUNK and cols > 0:
                break
            if runs and runs[-1][3] == d:
                runs[-1][2] += 1
            else:
                runs.append([cols, j, 1, d])
            cols += d
            j += 1
        ch = Plan()
        ch.node_off = i
        ch.n_nodes = j - i
        ch.cols = cols
        ch.cpad = _round_up(cols, 128)
        ch.runs = [tuple(r) for r in runs]
        chunks.append(ch)
        i = j
    pl.chunks = chunks
    pl.tot_cols = sum(ch.cpad for ch in chunks)

    # global column offset of every slot (accounting chunk padding)
    col_of_slot = np.zeros(pl.n_real_slots, np.int64)
    off = 0
    for ch in pl.chunks:
        sd = slot_d[ch.node_off: ch.node_off + ch.n_nodes]
        col_of_slot[ch.node_off: ch.node_off + ch.n_nodes] = (
            off + np.concatenate([[0], np.cumsum(sd)[:-1]]))
        off += ch.cpad

    # per-shard slot-expanded idx/attr arrays (vectorized)
    for sh in shards:
        idx = np.full(pl.tot_cols, pl.DUMTOK, np.int64)
        aidx = np.full(pl.tot_cols, -1, np.int64)  # original edge id or -1
        starts = np.zeros(sh.n_own + 1, np.int64)
        np.cumsum(sh.deg, out=starts[1:])
        rel = np.searchsorted(sh.ref, sh.src)  # relabeled src per edge
        npos = min(sh.n_own, pl.n_real_slots)
        nodes = sh.node_order[:npos]
        cnts = sh.deg[nodes]
        tot = int(cnts.sum())
        if tot:
            base = np.concatenate([[0], np.cumsum(cnts)[:-1]])
            within = np.arange(tot) - np.repeat(base, cnts)
            e_src = np.repeat(starts[nodes], cnts) + within
            c_dst = np.repeat(col_of_slot[:npos], cnts) + within
            idx[c_dst] = rel[e_src]
            aidx[c_dst] = sh.attr_eidx[e_src]
        sh.idx_flat = idx
        sh.attr_eid = aidx
    return pl


def build_inputs(pl, x, pre_h, edge_attr, weights):
    """Construct per-core in_maps (list of dicts) for run_bass_kernel_spmd."""
    bf16 = ml_dtypes.bfloat16
    (enc_w, enc_b, M_w, M_b, U_w, U_b, dec_w, dec_b) = weights
    n = pl.n_nodes

    pre_hT = np.ascontiguousarray(pre_h.T)           # [H, N] f32
    pre_hT_bf = pre_hT.astype(bf16)
    x_row = np.ascontiguousarray(x.reshape(1, -1))   # [1, N]
    x_row_bf = x_row.astype(bf16)
    attr_flat = np.asarray(edge_attr).reshape(-1)

    # shared weight tensors
    shared = {
        "enc_w_main_bf": enc_w[1:129].astype(bf16),          # [128,128]
        "enc_w_x_bf": enc_w[0:1].astype(bf16),               # [1,128]
        "enc_w_main_f": np.ascontiguousarray(enc_w[1:129], np.float32),
        "enc_w_x_f": np.ascontiguousarray(enc_w[0:1], np.float32),
        "enc_b": np.ascontiguousarray(enc_b.reshape(P, 1), np.float32),
        "m_src_bf": M_w[H:2 * H].astype(bf16),               # [128,128]
        "m_tgt_f": np.ascontiguousarray(M_w[:H], np.float32),
        "m_b_row": np.ascontiguousarray(M_b.reshape(1, H), np.float32),
        "w_a_col": np.ascontiguousarray(M_w[2 * H].reshape(P, 1), np.float32),
        "u_w1": np.ascontiguousarray(U_w[:H], np.float32),
        "u_w2": np.ascontiguousarray(U_w[H:], np.float32),
        "u_b_row": np.ascontiguousarray(U_b.reshape(1, H), np.float32),
        "dec_w1": np.ascontiguousarray(dec_w[:H], np.float32),   # [128,1]
        "dec_w2": np.ascontiguousarray(dec_w[H:], np.float32),
        "dec_b_row": np.ascontiguousarray(
            np.asarray(dec_b).reshape(1, 1), np.float32),
        "neg_row_bf": np.full((1, H), NEG, bf16),
    }

    in_maps = []
    for c in range(NCORES):
        m = dict(shared)
        ph_sh = np.zeros((NSUB, P, pl.T_TAB), bf16)
        x_sh = np.zeros((NSUB, 1, pl.T_TAB), bf16)
        idx_sh = np.zeros((NSUB, P, pl.tot_cols // 16), np.int16)
        attr_sh = np.zeros((NSUB, 32, pl.tot_cols), bf16)
        ph_own = np.zeros((NSUB, P, pl.S_OWN), np.float32)
        x_own = np.zeros((NSUB, 1, pl.S_OWN), np.float32)
        for k in range(NSUB):
            sh = pl.shards[c * NSUB + k]
            u = len(sh.ref)
            ph_sh[k, :, :u] = pre_hT_bf[:, sh.ref]
            x_sh[k, :, :u] = x_row_bf[:, sh.ref]
            i16 = sh.idx_flat.astype(np.int16)
            wrapped = i16.reshape(-1, 16).T                  # [16, tot/16]
            idx_sh[k] = np.tile(wrapped, (8, 1))
            av = np.where(sh.attr_eid >= 0,
                          attr_flat[np.maximum(sh.attr_eid, 0)], 0.0)
            attr_sh[k, :] = av.astype(bf16)[None, :]
            own_nodes = sh.node_order + sh.n0                # global ids
            ph_own[k, :, :sh.n_own] = pre_hT[:, own_nodes]
            x_own[k, :, :sh.n_own] = x_row[:, own_nodes]
        m["pre_h_shard"] = ph_sh
        m["x_shard"] = x_sh
        m["idx_buf"] = idx_sh
        m["attr_buf"] = attr_sh
        m["pre_h_own"] = ph_own
        m["x_own"] = x_own
        in_maps.append(m)
    return in_maps


# ----------------------------------------------------------------------------
# Device kernel
# ----------------------------------------------------------------------------

def build_nc(pl):
    import concourse.bass as bass
    import concourse.bacc as bacc
    import concourse.mybir as mybir
    import concourse.tile as tile

    dt = mybir.dt
    AF = mybir.ActivationFunctionType
    ALU = mybir.AluOpType

    nc = bacc.Bacc("TRN2", target_bir_lowering=False, debug=False,
                   enable_asserts=False, num_devices=NCORES)

    def din(name, shape, dtype):
        return nc.dram_tensor(name, list(shape), dtype,
                              kind="ExternalInput").ap()

    def dout(name, shape, dtype):
        return nc.dram_tensor(name, list(shape), dtype,
                              kind="ExternalOutput").ap()

    T, S = pl.T_TAB, pl.S_OWN
    TC = pl.tot_cols

    pre_h_shard = din("pre_h_shard", (NSUB, P, T), dt.bfloat16)
    x_shard = din("x_shard", (NSUB, 1, T), dt.bfloat16)
    idx_buf = din("idx_buf", (NSUB, P, TC // 16), dt.int16)
    attr_buf = din("attr_buf", (NSUB, 32, TC), dt.bfloat16)
    pre_h_own = din("pre_h_own", (NSUB, P, S), dt.float32)
    x_own = din("x_own", (NSUB, 1, S), dt.float32)

    enc_w_main_bf = din("enc_w_main_bf", (P, H), dt.bfloat16)
    enc_w_x_bf = din("enc_w_x_bf", (1, H), dt.bfloat16)
    enc_w_main_f = din("enc_w_main_f", (P, H), dt.float32)
    enc_w_x_f = din("enc_w_x_f", (1, H), dt.float32)
    enc_b = din("enc_b", (P, 1), dt.float32)
    m_src_bf = din("m_src_bf", (P, H), dt.bfloat16)
    m_tgt_f = din("m_tgt_f", (P, H), dt.float32)
    m_b_row = din("m_b_row", (1, H), dt.float32)
    w_a_col = din("w_a_col", (P, 1), dt.float32)
    u_w1 = din("u_w1", (P, H), dt.float32)
    u_w2 = din("u_w2", (P, H), dt.float32)
    u_b_row = din("u_b_row", (1, H), dt.float32)
    dec_w1 = din("dec_w1", (P, 1), dt.float32)
    dec_w2 = din("dec_w2", (P, 1), dt.float32)
    dec_b_row = din("dec_b_row", (1, 1), dt.float32)
    neg_row_bf = din("neg_row_bf", (1, H), dt.bfloat16)

    hT_out = dout("hT_out", (NSUB, P, S), dt.float32)
    yT_out = dout("yT_out", (NSUB, 1, S), dt.float32)

    skip = set(os.environ.get("KERNEL_SKIP", "").split(","))
    from contextlib import ExitStack
    with tile.TileContext(nc) as tc, ExitStack() as es:
        wpool = es.enter_context(tc.tile_pool(name="weights", bufs=1))
        dram = es.enter_context(tc.tile_pool(name="dram", bufs=1,
                                             space="DRAM"))
        p1 = es.enter_context(tc.tile_pool(name="p1", bufs=3))
        p1z = es.enter_context(tc.tile_pool(name="p1z", bufs=2))
        p1b = es.enter_context(tc.tile_pool(name="p1b", bufs=3))
        psz = es.enter_context(tc.tile_pool(name="psz", bufs=2, space="PSUM"))
        psb = es.enter_context(tc.tile_pool(name="psb", bufs=2, space="PSUM"))
        p2i = es.enter_context(tc.tile_pool(name="p2i", bufs=2))
        p2a = es.enter_context(tc.tile_pool(name="p2a", bufs=2))
        p2g = es.enter_context(tc.tile_pool(name="p2g", bufs=2))
        p2m = es.enter_context(tc.tile_pool(name="p2m", bufs=2))
        p2s = es.enter_context(tc.tile_pool(name="p2s", bufs=2))
        p3 = es.enter_context(tc.tile_pool(name="p3", bufs=2))
        ps3 = es.enter_context(tc.tile_pool(name="ps3", bufs=1, space="PSUM"))
        ps3y = es.enter_context(tc.tile_pool(name="ps3y", bufs=1,
                                             space="PSUM"))

        # --- load weights once ---
        def wtile(ap, shape, dtype):
            t = wpool.tile(list(shape), dtype, tag=ap.tensor.name)
            nc.sync.dma_start(out=t[:], in_=ap)
            return t

        w_enc_main_bf = wtile(enc_w_main_bf, (P, H), dt.bfloat16)
        w_enc_x_bf = wtile(enc_w_x_bf, (1, H), dt.bfloat16)
        w_enc_main_f = wtile(enc_w_main_f, (P, H), dt.float32)
        w_enc_x_f = wtile(enc_w_x_f, (1, H), dt.float32)
        w_enc_b = wtile(enc_b, (P, 1), dt.float32)
        w_m_src = wtile(m_src_bf, (P, H), dt.bfloat16)
        w_m_tgt = wtile(m_tgt_f, (P, H), dt.float32)
        w_m_b = wtile(m_b_row, (1, H), dt.float32)
        w_wa = wtile(w_a_col, (P, 1), dt.float32)
        w_u1 = wtile(u_w1, (P, H), dt.float32)
        w_u2 = wtile(u_w2, (P, H), dt.float32)
        w_ub = wtile(u_b_row, (1, H), dt.float32)
        w_d1 = wtile(dec_w1, (P, 1), dt.float32)
        w_d2 = wtile(dec_w2, (P, 1), dt.float32)
        w_db = wtile(dec_b_row, (1, 1), dt.float32)
        w_neg = wtile(neg_row_bf, (1, H), dt.bfloat16)

        ones = wpool.tile([1, 512], dt.float32, tag="ones")
        nc.vector.memset(ones[:], 1.0)

        tables = [dram.tile([T, H], dt.bfloat16, tag=f"table{s}",
                            name=f"table{s}") for s in range(NSUB)]
        segmaxs = [p2s.tile([P, S], dt.float32, tag=f"segmax{s}",
                            name=f"segmax{s}") for s in range(NSUB)]

        for s in range(NSUB):
            # ---------------- phase 1: build B table ----------------
            tab = tables[s]
            for g in range(T // 512) if "p1" not in skip else []:
                ph = p1.tile([P, 512], dt.bfloat16, tag="ph")
                nc.sync.dma_start(
                    out=ph[:], in_=pre_h_shard[s, :, g * 512:(g + 1) * 512])
                xx = p1.tile([1, 512], dt.bfloat16, tag="xx")
                nc.sync.dma_start(
                    out=xx[:], in_=x_shard[s, :, g * 512:(g + 1) * 512])
                zp = psz.tile([P, 512], dt.float32, space="PSUM")
                nc.tensor.matmul(zp[:], w_enc_main_bf[:], ph[:],
                                 start=True, stop=False)
                nc.tensor.matmul(zp[:], w_enc_x_bf[:], xx[:],
                                 start=False, stop=True)
                zt = p1z.tile([P, 512], dt.bfloat16, tag="zt")
                nc.scalar.activation(zt[:], zp[:], AF.Relu, bias=w_enc_b[:])
                for q in range(4):
                    bp = psb.tile([P, H], dt.float32, space="PSUM")
                    nc.tensor.matmul(bp[:], zt[:, q * 128:(q + 1) * 128],
                                     w_m_src[:], start=True, stop=True)
                    bs = p1b.tile([P, H], dt.bfloat16, tag="bs")
                    nc.scalar.copy(bs[:], bp[:])
                    r0 = g * 512 + q * 128
                    nc.sync.dma_start(out=tab[r0:r0 + 128, :], in_=bs[:])
            if "p1" not in skip:
                # dummy row <- NEG
                nc.sync.dma_start(out=tab[pl.DUMTOK:pl.DUMTOK + 1, :],
                                  in_=w_neg[:])

            # ---------------- phase 2: gather + segmented max --------
            seg = segmaxs[s]
            if "p2" in skip:
                nc.vector.memset(seg[:, :], 0.0)
            if pl.n_real_slots < S:
                nc.vector.memset(seg[:, pl.n_real_slots:], NEG)
            idxs = p2i.tile([P, TC // 16], dt.int16, tag="idxs")
            nc.sync.dma_start(out=idxs[:], in_=idx_buf[s])
            off = 0
            for ch in (pl.chunks if "p2" not in skip else []):
                at = p2a.tile([P, CCHUNK], dt.bfloat16, tag="attr")
                nc.sync.dma_start(out=at[0:32, :ch.cpad],
                                  in_=attr_buf[s, :, off:off + ch.cpad])
                nc.vector.tensor_copy(out=at[32:64, :ch.cpad],
                                      in_=at[0:32, :ch.cpad])
                nc.vector.tensor_copy(out=at[64:128, :ch.cpad],
                                      in_=at[0:64, :ch.cpad])
                gt = p2g.tile([P, CCHUNK], dt.bfloat16, tag="gath")
                if "gather" in skip:
                    nc.vector.memset(gt[:, :ch.cpad], 0.0)
                elif True:
                    nc.gpsimd.dma_gather(
                    out_ap=gt[:, :ch.cpad].rearrange("p (o c) -> p o c", o=1),
                    in_ap=tab[:, :],
                    idxs_ap=idxs[:, off // 16:(off + ch.cpad) // 16],
                        num_idxs=ch.cpad,
                        num_idxs_reg=ch.cpad,
                        elem_size=H,
                        transpose=True,
                        single_packet=False,
                    )
                if "stt" in skip:
                    pass
                else:
                    nc.vector.scalar_tensor_tensor(
                    out=gt[:, :ch.cpad], in0=at[:, :ch.cpad], scalar=w_wa[:],
                    in1=gt[:, :ch.cpad], op0=ALU.mult, op1=ALU.add)
                for (coff, noff, nn, d) in ch.runs:
                    nc.vector.reduce_max(
                        out=seg[:, noff:noff + nn],
                        in_=gt[:, coff:coff + nn * d].rearrange(
                            "p (n d) -> p n d", d=d),
                        axis=mybir.AxisListType.X)
                off += ch.cpad

            # ---------------- phase 3: node MLPs ---------------------
            for g in range(S // 512):
                sl = slice(g * 512, (g + 1) * 512)
                pho = p3.tile([P, 512], dt.float32, tag="pho")
                nc.sync.dma_start(out=pho[:], in_=pre_h_own[s, :, sl])
                xo = p3.tile([1, 512], dt.float32, tag="xo")
                nc.sync.dma_start(out=xo[:], in_=x_own[s, :, sl])
                zp = ps3.tile([P, 512], dt.float32, space="PSUM", tag="zp3")
                nc.tensor.matmul(zp[:], w_enc_main_f[:], pho[:],
                                 start=True, stop=False)
                nc.tensor.matmul(zp[:], w_enc_x_f[:], xo[:],
                                 start=False, stop=True)
                zo = p3.tile([P, 512], dt.float32, tag="zo")
                nc.scalar.activation(zo[:], zp[:], AF.Relu, bias=w_enc_b[:])

                ap_ = ps3.tile([P, 512], dt.float32, space="PSUM", tag="ap3")
                nc.tensor.matmul(ap_[:], w_m_tgt[:], zo[:],
                                 start=True, stop=False)
                nc.tensor.matmul(ap_[:], w_m_b[:], ones[:],
                                 start=False, stop=True)
                u = p3.tile([P, 512], dt.float32, tag="u")
                nc.vector.tensor_tensor(out=u[:], in0=ap_[:],
                                        in1=seg[:, sl], op=ALU.add)
                agg = p3.tile([P, 512], dt.float32, tag="agg")
                nc.scalar.activation(agg[:], u[:], AF.Relu)

                hp = ps3.tile([P, 512], dt.float32, space="PSUM", tag="hp3")
                nc.tensor.matmul(hp[:], w_u1[:], zo[:], start=True,
                                 stop=False)
                nc.tensor.matmul(hp[:], w_u2[:], agg[:], start=False,
                                 stop=False)
                nc.tensor.matmul(hp[:], w_ub[:], ones[:], start=False,
                                 stop=True)
                ho = p3.tile([P, 512], dt.float32, tag="ho")
                nc.scalar.activation(ho[:], hp[:], AF.Relu)
                nc.sync.dma_start(out=hT_out[s, :, sl], in_=ho[:])

                yp = ps3y.tile([1, 512], dt.float32, space="PSUM", tag="yp3")
                nc.tensor.matmul(yp[:], w_d1[:], zo[:], start=True,
                                 stop=False)
                nc.tensor.matmul(yp[:], w_d2[:], ho[:], start=False,
                                 stop=False)
                nc.tensor.matmul(yp[:], w_db[:], ones[:], start=False,
                                 stop=True)
                yo = p3.tile([1, 512], dt.float32, tag="yo")
                nc.scalar.activation(yo[:], yp[:], AF.Sigmoid)
                nc.sync.dma_start(out=yT_out[s, :, sl], in_=yo[:])

    nc.compile()
    return nc


# ----------------------------------------------------------------------------
# Public entry point
# ----------------------------------------------------------------------------

def _install_ntff_hook():
    """Make trace=True work under axon in images whose antenv lacks
    axon_hooks (degrades to no-trace on any failure)."""
    try:
        import types
        import antenv
        if "antenv.axon_hooks" not in sys.modules:
            mod = types.ModuleType("antenv.axon_hooks")
            _h = [None]
            mod.set_axon_ntff_profile_hook = lambda h: _h.__setitem__(0, h)
            mod.get_axon_ntff_profile_hook = lambda: _h[0]
            sys.modules["antenv.axon_hooks"] = mod
            antenv.axon_hooks = mod
        from antenv import axon_hooks
        if axon_hooks.get_axon_ntff_profile_hook() is None:
            from trn_agent_boot.trn_boot import _ntff_profile_via_ctypes
            axon_hooks.set_axon_ntff_profile_hook(
                _ntff_profile_via_ctypes("/opt/axon/libaxon_pjrt.so"))
    except Exception as e:  # noqa: BLE001
        print(f"ntff hook install failed ({e}); tracing disabled")


def kernel(x, pre_h, edge_attr, enc_w, enc_b, M_w, M_b, U_w, U_b,
           dec_w, dec_b, ter_w, ter_b, edge_index):
    from concourse import bass_utils
    trace = bool(int(os.environ.get("KERNEL_TRACE", "0")))
    if trace:
        _install_ntff_hook()

    x = np.asarray(x, np.float32)
    pre_h = np.asarray(pre_h, np.float32)
    edge_attr = np.asarray(edge_attr, np.float32)
    edge_index = np.asarray(edge_index)
    n_nodes = x.shape[0]

    pl = build_plan(edge_index, n_nodes)
    in_maps = build_inputs(pl, x, pre_h, edge_attr,
                           (np.asarray(enc_w, np.float32),
                            np.asarray(enc_b, np.float32),
                            np.asarray(M_w, np.float32),
                            np.asarray(M_b, np.float32),
                            np.asarray(U_w, np.float32),
                            np.asarray(U_b, np.float32),
                            np.asarray(dec_w, np.float32),
                            np.asarray(dec_b, np.float32)))
    nc = build_nc(pl)
    res = bass_utils.run_bass_kernel_spmd(
        nc, in_maps, core_ids=list(range(NCORES)), trace=trace)

    h = np.zeros((n_nodes, H), np.float32)
    y = np.zeros((n_nodes, 1), np.float32)
    for c in range(NCORES):
        out = res.results[c]
        for k in range(NSUB):
            sh = pl.shards[c * NSUB + k]
            nodes = sh.node_order + sh.n0
            h[nodes] = out["hT_out"][k, :, :sh.n_own].T
            y[nodes, 0] = out["yT_out"][k, 0, :sh.n_own]

    ter_w = np.asarray(ter_w, np.float32)
    hm = h.mean(0)
    ter = np.float32((hm @ (ter_w[:H, 0] + ter_w[H:, 0])) +
                     np.asarray(ter_b, np.float32).reshape(-1)[0])
    kernel.last_results = res
    return h, y, ter


# revision 21
# speedup vs baseline: 1.3183x; 1.3183x over previous
"""Trainium2 Bass kernel for BFS Neural Execution (GNN message passing).

Math (reference):
    z   = relu([x, pre_h] @ enc_w + enc_b)                         [N,H]
    m_e = relu([z[tgt_e], z[src_e], attr_e] @ M_w + M_b)           [E,H]
    agg = segment_max(m, tgt) (empty -> 0)                         [N,H]
    h   = relu([z, agg] @ U_w + U_b)                               [N,H]
    y   = sigmoid([z, h] @ dec_w + dec_b)                          [N,1]
    ter = mean([h, mean(h)] @ ter_w + ter_b)                       scalar

Refactoring used here: with A = z@M_w[:H]+M_b, B = z@M_w[H:2H], w_a = M_w[2H],
    m_e = relu(A[tgt] + B[src] + attr_e*w_a)
and since relu is monotone and A[tgt] is constant within a segment,
    agg[n] = relu(A[n] + max_{e->n}(B[src_e] + attr_e*w_a))
with empty segments handled by a -1e30 sentinel (relu(-huge) == 0).

Distribution: edges are sharded by contiguous target-node ranges into 16
sub-shards (2 per NeuronCore, processed sequentially).  Each sub-shard
relabels the source nodes it references (<=~27k distinct, fits int16) and the
device builds a bf16 B-table for exactly those nodes, then uses the custom
dma_gather (transpose mode) to fetch B rows feature-major, adds attr*w_a, and
does a segmented max via strided reduce_max over degree-padded slot groups.
The slot schedule (per-position padded degree) is shared across all 16
sub-shards so a single NEFF runs SPMD on all 8 cores.
"""

import math
import os
import sys

import numpy as np

for _p in ("/opt/trn_rl_repo", "/opt/trn_rl_repo/concourse"):
    if _p not in sys.path:
        sys.path.insert(0, _p)

import ml_dtypes

P = 128
H = 128
NCORES = 8
NSUB = 2              # sub-shards per core
NSHARD = NCORES * NSUB
CCHUNK = int(os.environ.get("KERNEL_CCHUNK", "8192"))
GATHER_SP = bool(int(os.environ.get("KERNEL_SP", "0")))
NEG = -1.0e30
# Allowed padded degrees (slot sizes).  Must be sorted ascending.
D_BUCKETS = [1, 2, 3, 4, 5, 6, 7, 8, 10, 12, 14, 16, 20, 24, 28, 32, 40, 48,
             64, 80, 96, 128, 192, 256, 384, 512, 768, 1024, 2048, 4096]


def _round_up(v, m):
    return -(-v // m) * m


def _bucket(d):
    if d == 0:
        return 0
    for b in D_BUCKETS:
        if b >= d:
            return b
    raise ValueError(f"degree {d} exceeds max bucket")


# ----------------------------------------------------------------------------
# Host-side planning
# ----------------------------------------------------------------------------

class Plan:
    pass


def build_plan(edge_index, n_nodes):
    """Shard edges by contiguous tgt ranges; build the shared slot schedule and
    per-shard index/relabel data."""
    src = np.asarray(edge_index[0], np.int64)
    tgt = np.asarray(edge_index[1], np.int64)
    E = src.shape[0]

    deg_all = np.bincount(tgt, minlength=n_nodes)
    cum = np.cumsum(deg_all)
    bounds = [0]
    for i in range(1, NSHARD):
        bounds.append(int(np.searchsorted(cum, E * i / NSHARD)))
    bounds.append(n_nodes)

    order = np.argsort(tgt, kind="stable")
    src_s, tgt_s = src[order], tgt[order]
    attr_perm = order  # edge permutation; attr gathered later

    # cut points in the sorted edge array per shard
    eb = [0]
    for i in range(1, NSHARD):
        eb.append(int(np.searchsorted(tgt_s, bounds[i])))
    eb.append(E)

    shards = []
    for s in range(NSHARD):
        sh = Plan()
        sh.n0, sh.n1 = bounds[s], bounds[s + 1]
        sh.e0, sh.e1 = eb[s], eb[s + 1]
        sh.src = src_s[sh.e0:sh.e1]
        sh.tgt = tgt_s[sh.e0:sh.e1]
        sh.attr_eidx = attr_perm[sh.e0:sh.e1]  # original edge ids
        sh.n_own = sh.n1 - sh.n0
        sh.deg = np.bincount(sh.tgt - sh.n0, minlength=sh.n_own)
        # node order: sorted by degree desc (stable)
        sh.node_order = np.argsort(-sh.deg, kind="stable")  # local ids
        sh.sorted_deg = sh.deg[sh.node_order]
        # distinct referenced srcs, ascending; relabel map
        sh.ref = np.unique(sh.src)
        shards.append(sh)

    pl = Plan()
    pl.n_nodes = n_nodes
    pl.E = E
    pl.shards = shards
    pl.S_OWN = _round_up(max(sh.n_own for sh in shards), 512)
    pl.T_TAB = _round_up(max(len(sh.ref) for sh in shards) + 1, 512)
    assert pl.T_TAB - 1 <= 32767, f"table too large for int16 idx: {pl.T_TAB}"
    pl.DUMTOK = pl.T_TAB - 1

    # shared slot schedule: profile[i] = max over shards of sorted_deg[i]
    prof = np.zeros(pl.S_OWN, np.int64)
    for sh in shards:
        prof[: sh.n_own] = np.maximum(prof[: sh.n_own], sh.sorted_deg)
    slot_d = np.array([_bucket(int(d)) for d in prof], np.int64)
    pl.slot_d = slot_d
    pl.n_real_slots = int((slot_d > 0).sum())  # trailing slots are d==0

    # chunking: group consecutive slots, <= CCHUNK padded columns per chunk
    chunks = []
    i = 0
    while i < pl.n_real_slots:
        cols = 0
        j = i
        runs = []  # [col_off, node_off, n_nodes, d]
        while j < pl.n_real_slots:
            d = int(slot_d[j])
            if cols + d > CCHUNK and cols > 0:
                break
            if runs and runs[-1][3] == d:
                runs[-1][2] += 1
            else:
                runs.append([cols, j, 1, d])
            cols += d
            j += 1
        ch = Plan()
        ch.node_off = i
        ch.n_nodes = j - i
        ch.cols = cols
        ch.cpad = _round_up(cols, 128)
        ch.runs = [tuple(r) for r in runs]
        chunks.append(ch)
        i = j
    pl.chunks = chunks
    pl.tot_cols = sum(ch.cpad for ch in chunks)

    # global column offset of every slot (accounting chunk padding)
    col_of_slot = np.zeros(pl.n_real_slots, np.int64)
    off = 0
    for ch in pl.chunks:
        sd = slot_d[ch.node_off: ch.node_off + ch.n_nodes]
        col_of_slot[ch.node_off: ch.node_off + ch.n_nodes] = (
            off + np.concatenate([[0], np.cumsum(sd)[:-1]]))
        off += ch.cpad

    # per-shard slot-expanded idx/attr arrays (vectorized)
    for sh in shards:
        idx = np.full(pl.tot_cols, pl.DUMTOK, np.int64)
        aidx = np.full(pl.tot_cols, -1, np.int64)  # original edge id or -1
        starts = np.zeros(sh.n_own + 1, np.int64)
        np.cumsum(sh.deg, out=starts[1:])
        rel = np.searchsorted(sh.ref, sh.src)  # relabeled src per edge
        npos = min(sh.n_own, pl.n_real_slots)
        nodes = sh.node_order[:npos]
        cnts = sh.deg[nodes]
        tot = int(cnts.sum())
        if tot:
            base = np.concatenate([[0], np.cumsum(cnts)[:-1]])
            within = np.arange(tot) - np.repeat(base, cnts)
            e_src = np.repeat(starts[nodes], cnts) + within
            c_dst = np.repeat(col_of_slot[:npos], cnts) + within
            idx[c_dst] = rel[e_src]
            aidx[c_dst] = sh.attr_eidx[e_src]
        sh.idx_flat = idx
        sh.attr_eid = aidx
    return pl


def build_inputs(pl, x, pre_h, edge_attr, weights):
    """Construct per-core in_maps (list of dicts) for run_bass_kernel_spmd."""
    bf16 = ml_dtypes.bfloat16
    (enc_w, enc_b, M_w, M_b, U_w, U_b, dec_w, dec_b) = weights
    n = pl.n_nodes

    pre_hT = np.ascontiguousarray(pre_h.T)           # [H, N] f32
    pre_hT_bf = pre_hT.astype(bf16)
    x_row = np.ascontiguousarray(x.reshape(1, -1))   # [1, N]
    x_row_bf = x_row.astype(bf16)
    attr_flat = np.asarray(edge_attr).reshape(-1)

    # shared weight tensors
    shared = {
        "enc_w_main_bf": enc_w[1:129].astype(bf16),          # [128,128]
        "enc_w_x_bf": enc_w[0:1].astype(bf16),               # [1,128]
        "enc_w_main_f": np.ascontiguousarray(enc_w[1:129], np.float32),
        "enc_w_x_f": np.ascontiguousarray(enc_w[0:1], np.float32),
        "enc_b": np.ascontiguousarray(enc_b.reshape(P, 1), np.float32),
        "m_src_bf": M_w[H:2 * H].astype(bf16),               # [128,128]
        "m_tgt_f": np.ascontiguousarray(M_w[:H], np.float32),
        "m_b_col": np.ascontiguousarray(M_b.reshape(P, 1), np.float32),
        "w_a_col": np.ascontiguousarray(M_w[2 * H].reshape(P, 1), np.float32),
        "u_w1": np.ascontiguousarray(U_w[:H], np.float32),
        "u_w2": np.ascontiguousarray(U_w[H:], np.float32),
        "u_b_col": np.ascontiguousarray(U_b.reshape(P, 1), np.float32),
        "dec_w1": np.ascontiguousarray(dec_w[:H], np.float32),   # [128,1]
        "dec_w2": np.ascontiguousarray(dec_w[H:], np.float32),
        "dec_b_sc": np.ascontiguousarray(
            np.asarray(dec_b).reshape(1, 1), np.float32),
        "neg_row_bf": np.full((1, H), NEG, bf16),
    }

    in_maps = []
    for c in range(NCORES):
        m = dict(shared)
        ph_sh = np.zeros((NSUB, P, pl.T_TAB), bf16)
        x_sh = np.zeros((NSUB, 1, pl.T_TAB), bf16)
        idx_sh = np.zeros((NSUB, P, pl.tot_cols // 16), np.int16)
        attr_sh = np.zeros((NSUB, P, pl.tot_cols), bf16)
        ph_own = np.zeros((NSUB, P, pl.S_OWN), np.float32)
        x_own = np.zeros((NSUB, 1, pl.S_OWN), np.float32)
        for k in range(NSUB):
            sh = pl.shards[c * NSUB + k]
            u = len(sh.ref)
            ph_sh[k, :, :u] = pre_hT_bf[:, sh.ref]
            x_sh[k, :, :u] = x_row_bf[:, sh.ref]
            i16 = sh.idx_flat.astype(np.int16)
            wrapped = i16.reshape(-1, 16).T                  # [16, tot/16]
            idx_sh[k] = np.tile(wrapped, (8, 1))
            av = np.where(sh.attr_eid >= 0,
                          attr_flat[np.maximum(sh.attr_eid, 0)], 0.0)
            attr_sh[k, :] = av.astype(bf16)[None, :]
            own_nodes = sh.node_order + sh.n0                # global ids
            ph_own[k, :, :sh.n_own] = pre_hT[:, own_nodes]
            x_own[k, :, :sh.n_own] = x_row[:, own_nodes]
        m["pre_h_shard"] = ph_sh
        m["x_shard"] = x_sh
        m["idx_buf"] = idx_sh
        m["attr_buf"] = attr_sh
        m["pre_h_own"] = ph_own
        m["x_own"] = x_own
        in_maps.append(m)
    return in_maps


# ----------------------------------------------------------------------------
# Device kernel
# ----------------------------------------------------------------------------

def build_nc(pl):
    import concourse.bass as bass
    import concourse.bacc as bacc
    import concourse.mybir as mybir
    import concourse.tile as tile

    dt = mybir.dt
    AF = mybir.ActivationFunctionType
    ALU = mybir.AluOpType

    nc = bacc.Bacc("TRN2", target_bir_lowering=False, debug=False,
                   enable_asserts=False, num_devices=NCORES)

    def din(name, shape, dtype):
        return nc.dram_tensor(name, list(shape), dtype,
                              kind="ExternalInput").ap()

    def dout(name, shape, dtype):
        return nc.dram_tensor(name, list(shape), dtype,
                              kind="ExternalOutput").ap()

    T, S = pl.T_TAB, pl.S_OWN
    TC = pl.tot_cols

    pre_h_shard = din("pre_h_shard", (NSUB, P, T), dt.bfloat16)
    x_shard = din("x_shard", (NSUB, 1, T), dt.bfloat16)
    idx_buf = din("idx_buf", (NSUB, P, TC // 16), dt.int16)
    attr_buf = din("attr_buf", (NSUB, P, TC), dt.bfloat16)
    pre_h_own = din("pre_h_own", (NSUB, P, S), dt.float32)
    x_own = din("x_own", (NSUB, 1, S), dt.float32)

    enc_w_main_bf = din("enc_w_main_bf", (P, H), dt.bfloat16)
    enc_w_x_bf = din("enc_w_x_bf", (1, H), dt.bfloat16)
    enc_w_main_f = din("enc_w_main_f", (P, H), dt.float32)
    enc_w_x_f = din("enc_w_x_f", (1, H), dt.float32)
    enc_b = din("enc_b", (P, 1), dt.float32)
    m_src_bf = din("m_src_bf", (P, H), dt.bfloat16)
    m_tgt_f = din("m_tgt_f", (P, H), dt.float32)
    m_b_col = din("m_b_col", (P, 1), dt.float32)
    w_a_col = din("w_a_col", (P, 1), dt.float32)
    u_w1 = din("u_w1", (P, H), dt.float32)
    u_w2 = din("u_w2", (P, H), dt.float32)
    u_b_col = din("u_b_col", (P, 1), dt.float32)
    dec_w1 = din("dec_w1", (P, 1), dt.float32)
    dec_w2 = din("dec_w2", (P, 1), dt.float32)
    dec_b_sc = din("dec_b_sc", (1, 1), dt.float32)
    neg_row_bf = din("neg_row_bf", (1, H), dt.bfloat16)

    hT_out = dout("hT_out", (NSUB, P, S), dt.float32)
    yT_out = dout("yT_out", (NSUB, 1, S), dt.float32)

    skip = set(os.environ.get("KERNEL_SKIP", "").split(","))
    from contextlib import ExitStack
    with tile.TileContext(nc) as tc, ExitStack() as es:
        wpool = es.enter_context(tc.tile_pool(name="weights", bufs=1))
        dram = es.enter_context(tc.tile_pool(name="dram", bufs=1,
                                             space="DRAM"))
        p1 = es.enter_context(tc.tile_pool(name="p1", bufs=3))
        p1z = es.enter_context(tc.tile_pool(name="p1z", bufs=2))
        p1b = es.enter_context(tc.tile_pool(name="p1b", bufs=3))
        psz = es.enter_context(tc.tile_pool(name="psz", bufs=2, space="PSUM"))
        psb = es.enter_context(tc.tile_pool(name="psb", bufs=2, space="PSUM"))
        p2i = es.enter_context(tc.tile_pool(name="p2i", bufs=2))
        p2a = es.enter_context(tc.tile_pool(name="p2a", bufs=2))
        p2g = es.enter_context(tc.tile_pool(name="p2g", bufs=2))
        p2s = es.enter_context(tc.tile_pool(name="p2s", bufs=2))
        p3 = es.enter_context(tc.tile_pool(name="p3", bufs=2))
        ps3 = es.enter_context(tc.tile_pool(name="ps3", bufs=1, space="PSUM"))
        ps3y = es.enter_context(tc.tile_pool(name="ps3y", bufs=1,
                                             space="PSUM"))

        def wtile(ap, shape, dtype):
            t = wpool.tile(list(shape), dtype, tag=ap.tensor.name,
                           name="w_" + ap.tensor.name)
            nc.sync.dma_start(out=t[:], in_=ap)
            return t

        w_enc_main_bf = wtile(enc_w_main_bf, (P, H), dt.bfloat16)
        w_enc_x_bf = wtile(enc_w_x_bf, (1, H), dt.bfloat16)
        w_enc_main_f = wtile(enc_w_main_f, (P, H), dt.float32)
        w_enc_x_f = wtile(enc_w_x_f, (1, H), dt.float32)
        w_enc_b = wtile(enc_b, (P, 1), dt.float32)
        w_m_src = wtile(m_src_bf, (P, H), dt.bfloat16)
        w_m_tgt = wtile(m_tgt_f, (P, H), dt.float32)
        w_m_b = wtile(m_b_col, (P, 1), dt.float32)
        w_wa = wtile(w_a_col, (P, 1), dt.float32)
        w_u1 = wtile(u_w1, (P, H), dt.float32)
        w_u2 = wtile(u_w2, (P, H), dt.float32)
        w_ub = wtile(u_b_col, (P, 1), dt.float32)
        w_d1 = wtile(dec_w1, (P, 1), dt.float32)
        w_d2 = wtile(dec_w2, (P, 1), dt.float32)
        w_db = wtile(dec_b_sc, (1, 1), dt.float32)
        w_neg = wtile(neg_row_bf, (1, H), dt.bfloat16)

        tables = [dram.tile([T, H], dt.bfloat16, tag=f"table{s}",
                            name=f"table{s}") for s in range(NSUB)]
        segmaxs = [p2s.tile([P, S], dt.float32, tag=f"segmax{s}",
                            name=f"segmax{s}") for s in range(NSUB)]
        idxtiles = [None] * NSUB

        # ---------------- emission helpers ----------------
        def emit_p1_group(s, g):
            tab = tables[s]
            ph = p1.tile([P, 512], dt.bfloat16, tag="ph", name="ph")
            nc.sync.dma_start(
                out=ph[:], in_=pre_h_shard[s, :, g * 512:(g + 1) * 512])
            xx = p1.tile([1, 512], dt.bfloat16, tag="xx", name="xx")
            nc.sync.dma_start(
                out=xx[:], in_=x_shard[s, :, g * 512:(g + 1) * 512])
            zp = psz.tile([P, 512], dt.float32, space="PSUM", name="zp")
            nc.tensor.matmul(zp[:], w_enc_main_bf[:], ph[:],
                             start=True, stop=False)
            nc.tensor.matmul(zp[:], w_enc_x_bf[:], xx[:],
                             start=False, stop=True)
            zt = p1z.tile([P, 512], dt.bfloat16, tag="zt", name="zt")
            nc.scalar.activation(zt[:], zp[:], AF.Relu, bias=w_enc_b[:])
            bp = psb.tile([P, 512], dt.float32, space="PSUM", name="bp")
            for q in range(4):
                nc.tensor.matmul(bp[:, q * 128:(q + 1) * 128],
                                 zt[:, q * 128:(q + 1) * 128],
                                 w_m_src[:], start=True, stop=True)
            bs = p1b.tile([P, 512], dt.bfloat16, tag="bs", name="bs")
            nc.scalar.copy(bs[:], bp[:])
            nc.sync.dma_start(
                out=tab[g * 512:(g + 1) * 512, :].rearrange(
                    "(q p) f -> p q f", p=128),
                in_=bs[:].rearrange("p (q f) -> p q f", q=4))

        def emit_p1_tail(s):
            if "p1" not in skip:
                nc.sync.dma_start(out=tables[s][pl.DUMTOK:pl.DUMTOK + 1, :],
                                  in_=w_neg[:])

        def emit_p2_head(s):
            seg = segmaxs[s]
            if "p2" in skip:
                nc.vector.memset(seg[:, :], 0.0)
            if pl.n_real_slots < S:
                nc.vector.memset(seg[:, pl.n_real_slots:], NEG)
            it = p2i.tile([P, TC // 16], dt.int16, tag="idxs", name="idxs")
            nc.scalar.dma_start(out=it[:], in_=idx_buf[s])
            idxtiles[s] = it

        def emit_p2_chunk(s, ch, off):
            seg = segmaxs[s]
            idxs = idxtiles[s]
            at = p2a.tile([P, CCHUNK], dt.bfloat16, tag="attr", name="at")
            nc.scalar.dma_start(out=at[:, :ch.cpad],
                                in_=attr_buf[s, :, off:off + ch.cpad])
            gt = p2g.tile([P, CCHUNK], dt.bfloat16, tag="gath", name="gt")
            if "gather" in skip:
                nc.vector.memset(gt[:, :ch.cpad], 0.0)
            else:
                nc.gpsimd.dma_gather(
                    out_ap=gt[:, :ch.cpad].rearrange("p (o c) -> p o c", o=1),
                    in_ap=tables[s][:, :],
                    idxs_ap=idxs[:, off // 16:(off + ch.cpad) // 16],
                    num_idxs=ch.cpad,
                    num_idxs_reg=ch.cpad,
                    elem_size=H,
                    transpose=True,
                    single_packet=GATHER_SP,
                )
            if "stt" not in skip:
                nc.vector.scalar_tensor_tensor(
                    out=gt[:, :ch.cpad], in0=at[:, :ch.cpad], scalar=w_wa[:],
                    in1=gt[:, :ch.cpad], op0=ALU.mult, op1=ALU.add)
            for (coff, noff, nn, d) in ch.runs:
                nc.vector.reduce_max(
                    out=seg[:, noff:noff + nn],
                    in_=gt[:, coff:coff + nn * d].rearrange(
                        "p (n d) -> p n d", d=d),
                    axis=mybir.AxisListType.X)

        def emit_p3_group(s, g):
            seg = segmaxs[s]
            sl = slice(g * 512, (g + 1) * 512)
            pho = p3.tile([P, 512], dt.float32, tag="pho", name="pho")
            nc.sync.dma_start(out=pho[:], in_=pre_h_own[s, :, sl])
            xo = p3.tile([1, 512], dt.float32, tag="xo", name="xo")
            nc.sync.dma_start(out=xo[:], in_=x_own[s, :, sl])
            zp = ps3.tile([P, 512], dt.float32, space="PSUM", tag="zp3",
                          name="zp3")
            nc.tensor.matmul(zp[:], w_enc_main_f[:], pho[:],
                             start=True, stop=False)
            nc.tensor.matmul(zp[:], w_enc_x_f[:], xo[:],
                             start=False, stop=True)
            zo = p3.tile([P, 512], dt.float32, tag="zo", name="zo")
            nc.scalar.activation(zo[:], zp[:], AF.Relu, bias=w_enc_b[:])

            ap_ = ps3.tile([P, 512], dt.float32, space="PSUM", tag="ap3",
                           name="ap3")
            nc.tensor.matmul(ap_[:], w_m_tgt[:], zo[:], start=True, stop=True)
            u = p3.tile([P, 512], dt.float32, tag="u", name="u")
            nc.vector.tensor_tensor(out=u[:], in0=ap_[:],
                                    in1=seg[:, sl], op=ALU.add)
            agg = p3.tile([P, 512], dt.float32, tag="agg", name="agg")
            nc.scalar.activation(agg[:], u[:], AF.Relu, bias=w_m_b[:])

            hp = ps3.tile([P, 512], dt.float32, space="PSUM", tag="hp3",
                          name="hp3")
            nc.tensor.matmul(hp[:], w_u1[:], zo[:], start=True, stop=False)
            nc.tensor.matmul(hp[:], w_u2[:], agg[:], start=False, stop=True)
            ho = p3.tile([P, 512], dt.float32, tag="ho", name="ho")
            nc.scalar.activation(ho[:], hp[:], AF.Relu, bias=w_ub[:])
            nc.sync.dma_start(out=hT_out[s, :, sl], in_=ho[:])

            yp = ps3y.tile([1, 512], dt.float32, space="PSUM", tag="yp3",
                           name="yp3")
            nc.tensor.matmul(yp[:], w_d1[:], zo[:], start=True, stop=False)
            nc.tensor.matmul(yp[:], w_d2[:], ho[:], start=False, stop=True)
            yo = p3.tile([1, 512], dt.float32, tag="yo", name="yo")
            nc.scalar.activation(yo[:], yp[:], AF.Sigmoid, bias=w_db[:])
            nc.sync.dma_start(out=yT_out[s, :, sl], in_=yo[:])

        # ---------------- emission schedule ----------------
        # p1(0); then p2(s) chunks interleaved with p1(s+1) groups; p3(s)
        # interleaved after.
        ngroups = T // 512 if "p1" not in skip else 0
        for g in range(ngroups):
            emit_p1_group(0, g)
        emit_p1_tail(0)
        for s in range(NSUB):
            emit_p2_head(s)
            nxt = list(range(ngroups)) if s + 1 < NSUB else []
            chunk_list = list(pl.chunks) if "p2" not in skip else []
            nch = max(len(chunk_list), 1)
            per = (len(nxt) + nch - 1) // nch if nxt else 0
            gi = 0
            off = 0
            for ci, ch in enumerate(chunk_list):
                emit_p2_chunk(s, ch, off)
                off += ch.cpad
                for _ in range(per):
                    if gi < len(nxt):
                        emit_p1_group(s + 1, nxt[gi])
                        gi += 1
            while gi < len(nxt):
                emit_p1_group(s + 1, nxt[gi])
                gi += 1
            if s + 1 < NSUB:
                emit_p1_tail(s + 1)
            for g in range(S // 512):
                emit_p3_group(s, g)

    nc.compile()
    return nc


# ----------------------------------------------------------------------------
# Public entry point
# ----------------------------------------------------------------------------

def _install_ntff_hook():
    """Make trace=True work under axon in images whose antenv lacks
    axon_hooks (degrades to no-trace on any failure)."""
    try:
        import types
        import antenv
        if "antenv.axon_hooks" not in sys.modules:
            mod = types.ModuleType("antenv.axon_hooks")
            _h = [None]
            mod.set_axon_ntff_profile_hook = lambda h: _h.__setitem__(0, h)
            mod.get_axon_ntff_profile_hook = lambda: _h[0]
            sys.modules["antenv.axon_hooks"] = mod
            antenv.axon_hooks = mod
        from antenv import axon_hooks
        if axon_hooks.get_axon_ntff_profile_hook() is None:
            from trn_agent_boot.trn_boot import _ntff_profile_via_ctypes
            axon_hooks.set_axon_ntff_profile_hook(
                _ntff_profile_via_ctypes("/opt/axon/libaxon_pjrt.so"))
    except Exception as e:  # noqa: BLE001
        print(f"ntff hook install failed ({e}); tracing disabled")


def kernel(x, pre_h, edge_attr, enc_w, enc_b, M_w, M_b, U_w, U_b,
           dec_w, dec_b, ter_w, ter_b, edge_index):
    from concourse import bass_utils
    trace = bool(int(os.environ.get("KERNEL_TRACE", "0")))
    if trace:
        _install_ntff_hook()

    x = np.asarray(x, np.float32)
    pre_h = np.asarray(pre_h, np.float32)
    edge_attr = np.asarray(edge_attr, np.float32)
    edge_index = np.asarray(edge_index)
    n_nodes = x.shape[0]

    pl = build_plan(edge_index, n_nodes)
    in_maps = build_inputs(pl, x, pre_h, edge_attr,
                           (np.asarray(enc_w, np.float32),
                            np.asarray(enc_b, np.float32),
                            np.asarray(M_w, np.float32),
                            np.asarray(M_b, np.float32),
                            np.asarray(U_w, np.float32),
                            np.asarray(U_b, np.float32),
                            np.asarray(dec_w, np.float32),
                            np.asarray(dec_b, np.float32)))
    nc = build_nc(pl)
    res = bass_utils.run_bass_kernel_spmd(
        nc, in_maps, core_ids=list(range(NCORES)), trace=trace)

    h = np.zeros((n_nodes, H), np.float32)
    y = np.zeros((n_nodes, 1), np.float32)
    for c in range(NCORES):
        out = res.results[c]
        for k in range(NSUB):
            sh = pl.shards[c * NSUB + k]
            nodes = sh.node_order + sh.n0
            h[nodes] = out["hT_out"][k, :, :sh.n_own].T
            y[nodes, 0] = out["yT_out"][k, 0, :sh.n_own]

    ter_w = np.asarray(ter_w, np.float32)
    hm = h.mean(0)
    ter = np.float32((hm @ (ter_w[:H, 0] + ter_w[H:, 0])) +
                     np.asarray(ter_b, np.float32).reshape(-1)[0])
    kernel.last_results = res
    return h, y, ter


# revision 22
# speedup vs baseline: 1.3677x; 1.0375x over previous
"""Trainium2 Bass kernel for BFS Neural Execution (GNN message passing).

Math (reference):
    z   = relu([x, pre_h] @ enc_w + enc_b)                         [N,H]
    m_e = relu([z[tgt_e], z[src_e], attr_e] @ M_w + M_b)           [E,H]
    agg = segment_max(m, tgt) (empty -> 0)                         [N,H]
    h   = relu([z, agg] @ U_w + U_b)                               [N,H]
    y   = sigmoid([z, h] @ dec_w + dec_b)                          [N,1]
    ter = mean([h, mean(h)] @ ter_w + ter_b)                       scalar

Refactoring used here: with A = z@M_w[:H]+M_b, B = z@M_w[H:2H], w_a = M_w[2H],
    m_e = relu(A[tgt] + B[src] + attr_e*w_a)
and since relu is monotone and A[tgt] is constant within a segment,
    agg[n] = relu(A[n] + max_{e->n}(B[src_e] + attr_e*w_a))
with empty segments handled by a -1e30 sentinel (relu(-huge) == 0).

Distribution: edges are sharded by contiguous target-node ranges into 16
sub-shards (2 per NeuronCore, processed sequentially).  Each sub-shard
relabels the source nodes it references (<=~27k distinct, fits int16) and the
device builds a bf16 B-table for exactly those nodes, then uses the custom
dma_gather (transpose mode) to fetch B rows feature-major, adds attr*w_a, and
does a segmented max via strided reduce_max over degree-padded slot groups.
The slot schedule (per-position padded degree) is shared across all 16
sub-shards so a single NEFF runs SPMD on all 8 cores.
"""

import math
import os
import sys

import numpy as np

for _p in ("/opt/trn_rl_repo", "/opt/trn_rl_repo/concourse"):
    if _p not in sys.path:
        sys.path.insert(0, _p)

import ml_dtypes

P = 128
H = 128
NCORES = 8
NSUB = 2              # sub-shards per core
NSHARD = NCORES * NSUB
CCHUNK = int(os.environ.get("KERNEL_CCHUNK", "8192"))
GATHER_SP = bool(int(os.environ.get("KERNEL_SP", "0")))
NEG = -1.0e30
# Allowed padded degrees (slot sizes).  Must be sorted ascending.
D_BUCKETS = (list(range(1, 65)) + [72, 80, 96, 112, 128, 160, 192, 256, 384,
                                   512, 768, 1024, 2048, 4096])


def _round_up(v, m):
    return -(-v // m) * m


def _bucket(d):
    if d == 0:
        return 0
    for b in D_BUCKETS:
        if b >= d:
            return b
    raise ValueError(f"degree {d} exceeds max bucket")


# ----------------------------------------------------------------------------
# Host-side planning
# ----------------------------------------------------------------------------

class Plan:
    pass


def build_plan(edge_index, n_nodes):
    """Shard edges by contiguous tgt ranges; build the shared slot schedule and
    per-shard index/relabel data."""
    src = np.asarray(edge_index[0], np.int64)
    tgt = np.asarray(edge_index[1], np.int64)
    E = src.shape[0]

    deg_all = np.bincount(tgt, minlength=n_nodes)
    cum = np.cumsum(deg_all)
    bounds = [0]
    for i in range(1, NSHARD):
        bounds.append(int(np.searchsorted(cum, E * i / NSHARD)))
    bounds.append(n_nodes)

    order = np.argsort(tgt, kind="stable")
    src_s, tgt_s = src[order], tgt[order]
    attr_perm = order  # edge permutation; attr gathered later

    # cut points in the sorted edge array per shard
    eb = [0]
    for i in range(1, NSHARD):
        eb.append(int(np.searchsorted(tgt_s, bounds[i])))
    eb.append(E)

    shards = []
    for s in range(NSHARD):
        sh = Plan()
        sh.n0, sh.n1 = bounds[s], bounds[s + 1]
        sh.e0, sh.e1 = eb[s], eb[s + 1]
        sh.src = src_s[sh.e0:sh.e1]
        sh.tgt = tgt_s[sh.e0:sh.e1]
        sh.attr_eidx = attr_perm[sh.e0:sh.e1]  # original edge ids
        sh.n_own = sh.n1 - sh.n0
        sh.deg = np.bincount(sh.tgt - sh.n0, minlength=sh.n_own)
        # node order: sorted by degree desc (stable)
        sh.node_order = np.argsort(-sh.deg, kind="stable")  # local ids
        sh.sorted_deg = sh.deg[sh.node_order]
        # distinct referenced srcs, ascending; relabel map
        sh.ref = np.unique(sh.src)
        shards.append(sh)

    pl = Plan()
    pl.n_nodes = n_nodes
    pl.E = E
    pl.shards = shards
    pl.S_OWN = _round_up(max(sh.n_own for sh in shards), 512)
    pl.T_TAB = _round_up(max(len(sh.ref) for sh in shards) + 1, 512)
    assert pl.T_TAB - 1 <= 32767, f"table too large for int16 idx: {pl.T_TAB}"
    pl.DUMTOK = pl.T_TAB - 1

    # shared slot schedule: profile[i] = max over shards of sorted_deg[i]
    prof = np.zeros(pl.S_OWN, np.int64)
    for sh in shards:
        prof[: sh.n_own] = np.maximum(prof[: sh.n_own], sh.sorted_deg)
    slot_d = np.array([_bucket(int(d)) for d in prof], np.int64)
    pl.slot_d = slot_d
    pl.n_real_slots = int((slot_d > 0).sum())  # trailing slots are d==0

    # chunking: group consecutive slots, <= CCHUNK padded columns per chunk
    chunks = []
    i = 0
    while i < pl.n_real_slots:
        cols = 0
        j = i
        runs = []  # [col_off, node_off, n_nodes, d]
        while j < pl.n_real_slots:
            d = int(slot_d[j])
            if cols + d > CCHUNK and cols > 0:
                break
            if runs and runs[-1][3] == d:
                runs[-1][2] += 1
            else:
                runs.append([cols, j, 1, d])
            cols += d
            j += 1
        ch = Plan()
        ch.node_off = i
        ch.n_nodes = j - i
        ch.cols = cols
        ch.cpad = _round_up(cols, 128)
        ch.runs = [tuple(r) for r in runs]
        chunks.append(ch)
        i = j
    pl.chunks = chunks
    pl.tot_cols = sum(ch.cpad for ch in chunks)

    # global column offset of every slot (accounting chunk padding)
    col_of_slot = np.zeros(pl.n_real_slots, np.int64)
    off = 0
    for ch in pl.chunks:
        sd = slot_d[ch.node_off: ch.node_off + ch.n_nodes]
        col_of_slot[ch.node_off: ch.node_off + ch.n_nodes] = (
            off + np.concatenate([[0], np.cumsum(sd)[:-1]]))
        off += ch.cpad

    # per-shard slot-expanded idx/attr arrays (vectorized)
    for sh in shards:
        idx = np.full(pl.tot_cols, pl.DUMTOK, np.int64)
        aidx = np.full(pl.tot_cols, -1, np.int64)  # original edge id or -1
        starts = np.zeros(sh.n_own + 1, np.int64)
        np.cumsum(sh.deg, out=starts[1:])
        rel = np.searchsorted(sh.ref, sh.src)  # relabeled src per edge
        npos = min(sh.n_own, pl.n_real_slots)
        nodes = sh.node_order[:npos]
        cnts = sh.deg[nodes]
        tot = int(cnts.sum())
        if tot:
            base = np.concatenate([[0], np.cumsum(cnts)[:-1]])
            within = np.arange(tot) - np.repeat(base, cnts)
            e_src = np.repeat(starts[nodes], cnts) + within
            c_dst = np.repeat(col_of_slot[:npos], cnts) + within
            idx[c_dst] = rel[e_src]
            aidx[c_dst] = sh.attr_eidx[e_src]
        sh.idx_flat = idx
        sh.attr_eid = aidx
    return pl


def build_inputs(pl, x, pre_h, edge_attr, weights):
    """Construct per-core in_maps (list of dicts) for run_bass_kernel_spmd."""
    bf16 = ml_dtypes.bfloat16
    (enc_w, enc_b, M_w, M_b, U_w, U_b, dec_w, dec_b) = weights
    n = pl.n_nodes

    pre_hT = np.ascontiguousarray(pre_h.T)           # [H, N] f32
    pre_hT_bf = pre_hT.astype(bf16)
    x_row = np.ascontiguousarray(x.reshape(1, -1))   # [1, N]
    x_row_bf = x_row.astype(bf16)
    attr_flat = np.asarray(edge_attr).reshape(-1)

    # shared weight tensors
    shared = {
        "enc_w_main_bf": enc_w[1:129].astype(bf16),          # [128,128]
        "enc_w_x_bf": enc_w[0:1].astype(bf16),               # [1,128]
        "enc_w_main_f": np.ascontiguousarray(enc_w[1:129], np.float32),
        "enc_w_x_f": np.ascontiguousarray(enc_w[0:1], np.float32),
        "enc_b": np.ascontiguousarray(enc_b.reshape(P, 1), np.float32),
        "m_src_bf": M_w[H:2 * H].astype(bf16),               # [128,128]
        "m_tgt_f": np.ascontiguousarray(M_w[:H], np.float32),
        "m_b_col": np.ascontiguousarray(M_b.reshape(P, 1), np.float32),
        "w_a_col": np.ascontiguousarray(M_w[2 * H].reshape(P, 1), np.float32),
        "u_w1": np.ascontiguousarray(U_w[:H], np.float32),
        "u_w2": np.ascontiguousarray(U_w[H:], np.float32),
        "u_b_col": np.ascontiguousarray(U_b.reshape(P, 1), np.float32),
        "dec_w1": np.ascontiguousarray(dec_w[:H], np.float32),   # [128,1]
        "dec_w2": np.ascontiguousarray(dec_w[H:], np.float32),
        "dec_b_sc": np.ascontiguousarray(
            np.asarray(dec_b).reshape(1, 1), np.float32),
        "neg_row_bf": np.full((1, H), NEG, bf16),
    }

    in_maps = []
    for c in range(NCORES):
        m = dict(shared)
        ph_sh = np.zeros((NSUB, P, pl.T_TAB), bf16)
        x_sh = np.zeros((NSUB, 1, pl.T_TAB), bf16)
        idx_sh = np.zeros((NSUB, P, pl.tot_cols // 16), np.int16)
        attr_sh = np.zeros((NSUB, P, pl.tot_cols), bf16)
        ph_own = np.zeros((NSUB, P, pl.S_OWN), np.float32)
        x_own = np.zeros((NSUB, 1, pl.S_OWN), np.float32)
        for k in range(NSUB):
            sh = pl.shards[c * NSUB + k]
            u = len(sh.ref)
            ph_sh[k, :, :u] = pre_hT_bf[:, sh.ref]
            x_sh[k, :, :u] = x_row_bf[:, sh.ref]
            i16 = sh.idx_flat.astype(np.int16)
            wrapped = i16.reshape(-1, 16).T                  # [16, tot/16]
            idx_sh[k] = np.tile(wrapped, (8, 1))
            av = np.where(sh.attr_eid >= 0,
                          attr_flat[np.maximum(sh.attr_eid, 0)], 0.0)
            attr_sh[k, :] = av.astype(bf16)[None, :]
            own_nodes = sh.node_order + sh.n0                # global ids
            ph_own[k, :, :sh.n_own] = pre_hT[:, own_nodes]
            x_own[k, :, :sh.n_own] = x_row[:, own_nodes]
        m["pre_h_shard"] = ph_sh
        m["x_shard"] = x_sh
        m["idx_buf"] = idx_sh
        m["attr_buf"] = attr_sh
        m["pre_h_own"] = ph_own
        m["x_own"] = x_own
        in_maps.append(m)
    return in_maps


# ----------------------------------------------------------------------------
# Device kernel
# ----------------------------------------------------------------------------

def build_nc(pl):
    import concourse.bass as bass
    import concourse.bacc as bacc
    import concourse.mybir as mybir
    import concourse.tile as tile

    dt = mybir.dt
    AF = mybir.ActivationFunctionType
    ALU = mybir.AluOpType

    nc = bacc.Bacc("TRN2", target_bir_lowering=False, debug=False,
                   enable_asserts=False, num_devices=NCORES)

    def din(name, shape, dtype):
        return nc.dram_tensor(name, list(shape), dtype,
                              kind="ExternalInput").ap()

    def dout(name, shape, dtype):
        return nc.dram_tensor(name, list(shape), dtype,
                              kind="ExternalOutput").ap()

    T, S = pl.T_TAB, pl.S_OWN
    TC = pl.tot_cols

    pre_h_shard = din("pre_h_shard", (NSUB, P, T), dt.bfloat16)
    x_shard = din("x_shard", (NSUB, 1, T), dt.bfloat16)
    idx_buf = din("idx_buf", (NSUB, P, TC // 16), dt.int16)
    attr_buf = din("attr_buf", (NSUB, P, TC), dt.bfloat16)
    pre_h_own = din("pre_h_own", (NSUB, P, S), dt.float32)
    x_own = din("x_own", (NSUB, 1, S), dt.float32)

    enc_w_main_bf = din("enc_w_main_bf", (P, H), dt.bfloat16)
    enc_w_x_bf = din("enc_w_x_bf", (1, H), dt.bfloat16)
    enc_w_main_f = din("enc_w_main_f", (P, H), dt.float32)
    enc_w_x_f = din("enc_w_x_f", (1, H), dt.float32)
    enc_b = din("enc_b", (P, 1), dt.float32)
    m_src_bf = din("m_src_bf", (P, H), dt.bfloat16)
    m_tgt_f = din("m_tgt_f", (P, H), dt.float32)
    m_b_col = din("m_b_col", (P, 1), dt.float32)
    w_a_col = din("w_a_col", (P, 1), dt.float32)
    u_w1 = din("u_w1", (P, H), dt.float32)
    u_w2 = din("u_w2", (P, H), dt.float32)
    u_b_col = din("u_b_col", (P, 1), dt.float32)
    dec_w1 = din("dec_w1", (P, 1), dt.float32)
    dec_w2 = din("dec_w2", (P, 1), dt.float32)
    dec_b_sc = din("dec_b_sc", (1, 1), dt.float32)
    neg_row_bf = din("neg_row_bf", (1, H), dt.bfloat16)

    hT_out = dout("hT_out", (NSUB, P, S), dt.float32)
    yT_out = dout("yT_out", (NSUB, 1, S), dt.float32)

    skip = set(os.environ.get("KERNEL_SKIP", "").split(","))
    from contextlib import ExitStack
    with tile.TileContext(nc) as tc, ExitStack() as es:
        wpool = es.enter_context(tc.tile_pool(name="weights", bufs=1))
        dram = es.enter_context(tc.tile_pool(name="dram", bufs=1,
                                             space="DRAM"))
        p1 = es.enter_context(tc.tile_pool(name="p1", bufs=3))
        p1z = es.enter_context(tc.tile_pool(name="p1z", bufs=2))
        p1b = es.enter_context(tc.tile_pool(name="p1b", bufs=3))
        psz = es.enter_context(tc.tile_pool(name="psz", bufs=2, space="PSUM"))
        psb = es.enter_context(tc.tile_pool(name="psb", bufs=2, space="PSUM"))
        p2i = es.enter_context(tc.tile_pool(name="p2i", bufs=2))
        p2a = es.enter_context(tc.tile_pool(name="p2a", bufs=2))
        p2g = es.enter_context(tc.tile_pool(name="p2g", bufs=2))
        p2s = es.enter_context(tc.tile_pool(name="p2s", bufs=2))
        p3 = es.enter_context(tc.tile_pool(name="p3", bufs=2))
        ps3 = es.enter_context(tc.tile_pool(name="ps3", bufs=1, space="PSUM"))
        ps3y = es.enter_context(tc.tile_pool(name="ps3y", bufs=1,
                                             space="PSUM"))

        def wtile(ap, shape, dtype):
            t = wpool.tile(list(shape), dtype, tag=ap.tensor.name,
                           name="w_" + ap.tensor.name)
            nc.sync.dma_start(out=t[:], in_=ap)
            return t

        w_enc_main_bf = wtile(enc_w_main_bf, (P, H), dt.bfloat16)
        w_enc_x_bf = wtile(enc_w_x_bf, (1, H), dt.bfloat16)
        w_enc_main_f = wtile(enc_w_main_f, (P, H), dt.float32)
        w_enc_x_f = wtile(enc_w_x_f, (1, H), dt.float32)
        w_enc_b = wtile(enc_b, (P, 1), dt.float32)
        w_m_src = wtile(m_src_bf, (P, H), dt.bfloat16)
        w_m_tgt = wtile(m_tgt_f, (P, H), dt.float32)
        w_m_b = wtile(m_b_col, (P, 1), dt.float32)
        w_wa = wtile(w_a_col, (P, 1), dt.float32)
        w_u1 = wtile(u_w1, (P, H), dt.float32)
        w_u2 = wtile(u_w2, (P, H), dt.float32)
        w_ub = wtile(u_b_col, (P, 1), dt.float32)
        w_d1 = wtile(dec_w1, (P, 1), dt.float32)
        w_d2 = wtile(dec_w2, (P, 1), dt.float32)
        w_db = wtile(dec_b_sc, (1, 1), dt.float32)
        w_neg = wtile(neg_row_bf, (1, H), dt.bfloat16)

        tables = [dram.tile([T, H], dt.bfloat16, tag=f"table{s}",
                            name=f"table{s}") for s in range(NSUB)]
        segmaxs = [p2s.tile([P, S], dt.float32, tag=f"segmax{s}",
                            name=f"segmax{s}") for s in range(NSUB)]
        idxtiles = [None] * NSUB

        # ---------------- emission helpers ----------------
        def emit_p1_group(s, g):
            tab = tables[s]
            ph = p1.tile([P, 512], dt.bfloat16, tag="ph", name="ph")
            nc.sync.dma_start(
                out=ph[:], in_=pre_h_shard[s, :, g * 512:(g + 1) * 512])
            xx = p1.tile([1, 512], dt.bfloat16, tag="xx", name="xx")
            nc.sync.dma_start(
                out=xx[:], in_=x_shard[s, :, g * 512:(g + 1) * 512])
            zp = psz.tile([P, 512], dt.float32, space="PSUM", name="zp")
            nc.tensor.matmul(zp[:], w_enc_main_bf[:], ph[:],
                             start=True, stop=False)
            nc.tensor.matmul(zp[:], w_enc_x_bf[:], xx[:],
                             start=False, stop=True)
            zt = p1z.tile([P, 512], dt.bfloat16, tag="zt", name="zt")
            nc.scalar.activation(zt[:], zp[:], AF.Relu, bias=w_enc_b[:])
            bp = psb.tile([P, 512], dt.float32, space="PSUM", name="bp")
            for q in range(4):
                nc.tensor.matmul(bp[:, q * 128:(q + 1) * 128],
                                 zt[:, q * 128:(q + 1) * 128],
                                 w_m_src[:], start=True, stop=True)
            bs = p1b.tile([P, 512], dt.bfloat16, tag="bs", name="bs")
            nc.scalar.copy(bs[:], bp[:])
            nc.sync.dma_start(
                out=tab[g * 512:(g + 1) * 512, :].rearrange(
                    "(q p) f -> p q f", p=128),
                in_=bs[:].rearrange("p (q f) -> p q f", q=4))

        def emit_p1_tail(s):
            if "p1" not in skip:
                nc.sync.dma_start(out=tables[s][pl.DUMTOK:pl.DUMTOK + 1, :],
                                  in_=w_neg[:])

        def emit_p2_head(s):
            seg = segmaxs[s]
            if "p2" in skip:
                nc.vector.memset(seg[:, :], 0.0)
            if pl.n_real_slots < S:
                nc.vector.memset(seg[:, pl.n_real_slots:], NEG)
            it = p2i.tile([P, TC // 16], dt.int16, tag="idxs", name="idxs")
            nc.scalar.dma_start(out=it[:], in_=idx_buf[s])
            idxtiles[s] = it

        def emit_p2_chunk(s, ch, off):
            seg = segmaxs[s]
            idxs = idxtiles[s]
            at = p2a.tile([P, CCHUNK], dt.bfloat16, tag="attr", name="at")
            nc.scalar.dma_start(out=at[:, :ch.cpad],
                                in_=attr_buf[s, :, off:off + ch.cpad])
            gt = p2g.tile([P, CCHUNK], dt.bfloat16, tag="gath", name="gt")
            if "gather" in skip:
                nc.vector.memset(gt[:, :ch.cpad], 0.0)
            else:
                nc.gpsimd.dma_gather(
                    out_ap=gt[:, :ch.cpad].rearrange("p (o c) -> p o c", o=1),
                    in_ap=tables[s][:, :],
                    idxs_ap=idxs[:, off // 16:(off + ch.cpad) // 16],
                    num_idxs=ch.cpad,
                    num_idxs_reg=ch.cpad,
                    elem_size=H,
                    transpose=True,
                    single_packet=GATHER_SP,
                )
            if "stt" not in skip:
                nc.vector.scalar_tensor_tensor(
                    out=gt[:, :ch.cpad], in0=at[:, :ch.cpad], scalar=w_wa[:],
                    in1=gt[:, :ch.cpad], op0=ALU.mult, op1=ALU.add)
            for (coff, noff, nn, d) in ch.runs:
                nc.vector.reduce_max(
                    out=seg[:, noff:noff + nn],
                    in_=gt[:, coff:coff + nn * d].rearrange(
                        "p (n d) -> p n d", d=d),
                    axis=mybir.AxisListType.X)

        def emit_p3_group(s, g):
            seg = segmaxs[s]
            sl = slice(g * 512, (g + 1) * 512)
            pho = p3.tile([P, 512], dt.float32, tag="pho", name="pho")
            nc.sync.dma_start(out=pho[:], in_=pre_h_own[s, :, sl])
            xo = p3.tile([1, 512], dt.float32, tag="xo", name="xo")
            nc.sync.dma_start(out=xo[:], in_=x_own[s, :, sl])
            zp = ps3.tile([P, 512], dt.float32, space="PSUM", tag="zp3",
                          name="zp3")
            nc.tensor.matmul(zp[:], w_enc_main_f[:], pho[:],
                             start=True, stop=False)
            nc.tensor.matmul(zp[:], w_enc_x_f[:], xo[:],
                             start=False, stop=True)
            zo = p3.tile([P, 512], dt.float32, tag="zo", name="zo")
            nc.scalar.activation(zo[:], zp[:], AF.Relu, bias=w_enc_b[:])

            ap_ = ps3.tile([P, 512], dt.float32, space="PSUM", tag="ap3",
                           name="ap3")
            nc.tensor.matmul(ap_[:], w_m_tgt[:], zo[:], start=True, stop=True)
            u = p3.tile([P, 512], dt.float32, tag="u", name="u")
            nc.vector.tensor_tensor(out=u[:], in0=ap_[:],
                                    in1=seg[:, sl], op=ALU.add)
            agg = p3.tile([P, 512], dt.float32, tag="agg", name="agg")
            nc.scalar.activation(agg[:], u[:], AF.Relu, bias=w_m_b[:])

            hp = ps3.tile([P, 512], dt.float32, space="PSUM", tag="hp3",
                          name="hp3")
            nc.tensor.matmul(hp[:], w_u1[:], zo[:], start=True, stop=False)
            nc.tensor.matmul(hp[:], w_u2[:], agg[:], start=False, stop=True)
            ho = p3.tile([P, 512], dt.float32, tag="ho", name="ho")
            nc.scalar.activation(ho[:], hp[:], AF.Relu, bias=w_ub[:])
            nc.sync.dma_start(out=hT_out[s, :, sl], in_=ho[:])

            yp = ps3y.tile([1, 512], dt.float32, space="PSUM", tag="yp3",
                           name="yp3")
            nc.tensor.matmul(yp[:], w_d1[:], zo[:], start=True, stop=False)
            nc.tensor.matmul(yp[:], w_d2[:], ho[:], start=False, stop=True)
            yo = p3.tile([1, 512], dt.float32, tag="yo", name="yo")
            nc.scalar.activation(yo[:], yp[:], AF.Sigmoid, bias=w_db[:])
            nc.sync.dma_start(out=yT_out[s, :, sl], in_=yo[:])

        # ---------------- emission schedule ----------------
        # p1(0); then p2(s) chunks interleaved with p1(s+1) groups; p3(s)
        # interleaved after.
        ngroups = T // 512 if "p1" not in skip else 0
        for g in range(ngroups):
            emit_p1_group(0, g)
        emit_p1_tail(0)
        for s in range(NSUB):
            emit_p2_head(s)
            nxt = list(range(ngroups)) if s + 1 < NSUB else []
            chunk_list = list(pl.chunks) if "p2" not in skip else []
            nch = max(len(chunk_list), 1)
            per = (len(nxt) + nch - 1) // nch if nxt else 0
            gi = 0
            off = 0
            for ci, ch in enumerate(chunk_list):
                emit_p2_chunk(s, ch, off)
                off += ch.cpad
                for _ in range(per):
                    if gi < len(nxt):
                        emit_p1_group(s + 1, nxt[gi])
                        gi += 1
            while gi < len(nxt):
                emit_p1_group(s + 1, nxt[gi])
                gi += 1
            if s + 1 < NSUB:
                emit_p1_tail(s + 1)
            for g in range(S // 512):
                emit_p3_group(s, g)

    nc.compile()
    return nc


# ----------------------------------------------------------------------------
# Public entry point
# ----------------------------------------------------------------------------

def _install_ntff_hook():
    """Make trace=True work under axon in images whose antenv lacks
    axon_hooks (degrades to no-trace on any failure)."""
    try:
        import types
        import antenv
        if "antenv.axon_hooks" not in sys.modules:
            mod = types.ModuleType("antenv.axon_hooks")
            _h = [None]
            mod.set_axon_ntff_profile_hook = lambda h: _h.__setitem__(0, h)
            mod.get_axon_ntff_profile_hook = lambda: _h[0]
            sys.modules["antenv.axon_hooks"] = mod
            antenv.axon_hooks = mod
        from antenv import axon_hooks
        if axon_hooks.get_axon_ntff_profile_hook() is None:
            from trn_agent_boot.trn_boot import _ntff_profile_via_ctypes
            axon_hooks.set_axon_ntff_profile_hook(
                _ntff_profile_via_ctypes("/opt/axon/libaxon_pjrt.so"))
    except Exception as e:  # noqa: BLE001
        print(f"ntff hook install failed ({e}); tracing disabled")


def kernel(x, pre_h, edge_attr, enc_w, enc_b, M_w, M_b, U_w, U_b,
           dec_w, dec_b, ter_w, ter_b, edge_index):
    from concourse import bass_utils
    trace = bool(int(os.environ.get("KERNEL_TRACE", "0")))
    if trace:
        _install_ntff_hook()

    x = np.asarray(x, np.float32)
    pre_h = np.asarray(pre_h, np.float32)
    edge_attr = np.asarray(edge_attr, np.float32)
    edge_index = np.asarray(edge_index)
    n_nodes = x.shape[0]

    pl = build_plan(edge_index, n_nodes)
    in_maps = build_inputs(pl, x, pre_h, edge_attr,
                           (np.asarray(enc_w, np.float32),
                            np.asarray(enc_b, np.float32),
                            np.asarray(M_w, np.float32),
                            np.asarray(M_b, np.float32),
                            np.asarray(U_w, np.float32),
                            np.asarray(U_b, np.float32),
                            np.asarray(dec_w, np.float32),
                            np.asarray(dec_b, np.float32)))
    nc = build_nc(pl)
    res = bass_utils.run_bass_kernel_spmd(
        nc, in_maps, core_ids=list(range(NCORES)), trace=trace)

    h = np.zeros((n_nodes, H), np.float32)
    y = np.zeros((n_nodes, 1), np.float32)
    for c in range(NCORES):
        out = res.results[c]
        for k in range(NSUB):
            sh = pl.shards[c * NSUB + k]
            nodes = sh.node_order + sh.n0
            h[nodes] = out["hT_out"][k, :, :sh.n_own].T
            y[nodes, 0] = out["yT_out"][k, 0, :sh.n_own]

    ter_w = np.asarray(ter_w, np.float32)
    hm = h.mean(0)
    ter = np.float32((hm @ (ter_w[:H, 0] + ter_w[H:, 0])) +
                     np.asarray(ter_b, np.float32).reshape(-1)[0])
    kernel.last_results = res
    return h, y, ter


# revision 26
# speedup vs baseline: 1.5234x; 1.1138x over previous
"""Trainium2 Bass kernel for BFS Neural Execution (GNN message passing).

Math (reference):
    z   = relu([x, pre_h] @ enc_w + enc_b)                         [N,H]
    m_e = relu([z[tgt_e], z[src_e], attr_e] @ M_w + M_b)           [E,H]
    agg = segment_max(m, tgt) (empty -> 0)                         [N,H]
    h   = relu([z, agg] @ U_w + U_b)                               [N,H]
    y   = sigmoid([z, h] @ dec_w + dec_b)                          [N,1]
    ter = mean([h, mean(h)] @ ter_w + ter_b)                       scalar

Refactoring used here: with A = z@M_w[:H]+M_b, B = z@M_w[H:2H], w_a = M_w[2H],
    m_e = relu(A[tgt] + B[src] + attr_e*w_a)
and since relu is monotone and A[tgt] is constant within a segment,
    agg[n] = relu(A[n] + max_{e->n}(B[src_e] + attr_e*w_a))
with empty segments handled by a -1e30 sentinel (relu(-huge) == 0).

Distribution: edges are sharded by contiguous target-node ranges into 16
sub-shards (2 per NeuronCore, processed sequentially).  Each sub-shard
relabels the source nodes it references (<=~27k distinct, fits int16) and the
device builds a bf16 B-table for exactly those nodes, then uses the custom
dma_gather (transpose mode) to fetch B rows feature-major, adds attr*w_a, and
does a segmented max via strided reduce_max over degree-padded slot groups.
The slot schedule (per-position padded degree) is shared across all 16
sub-shards so a single NEFF runs SPMD on all 8 cores.
"""

import math
import os
import sys

import numpy as np

for _p in ("/opt/trn_rl_repo", "/opt/trn_rl_repo/concourse"):
    if _p not in sys.path:
        sys.path.insert(0, _p)

import ml_dtypes

P = 128
H = 128
NCORES = 8
NSUB = 2              # sub-shards per core
NSHARD = NCORES * NSUB
CCHUNK = int(os.environ.get("KERNEL_CCHUNK", "8192"))
GATHER_SP = bool(int(os.environ.get("KERNEL_SP", "0")))
NEG = -1.0e30
# Allowed padded degrees (slot sizes).  Must be sorted ascending.
D_BUCKETS = (list(range(1, 65)) + [72, 80, 96, 112, 128, 160, 192, 256, 384,
                                   512, 768, 1024, 2048, 4096])


def _round_up(v, m):
    return -(-v // m) * m


def _bucket(d):
    if d == 0:
        return 0
    for b in D_BUCKETS:
        if b >= d:
            return b
    raise ValueError(f"degree {d} exceeds max bucket")


# ----------------------------------------------------------------------------
# Host-side planning
# ----------------------------------------------------------------------------

class Plan:
    pass


def build_plan(edge_index, n_nodes):
    """Shard edges by contiguous tgt ranges; build the shared slot schedule and
    per-shard index/relabel data."""
    src = np.asarray(edge_index[0], np.int64)
    tgt = np.asarray(edge_index[1], np.int64)
    E = src.shape[0]

    deg_all = np.bincount(tgt, minlength=n_nodes)
    cum = np.cumsum(deg_all)
    bounds = [0]
    for i in range(1, NSHARD):
        bounds.append(int(np.searchsorted(cum, E * i / NSHARD)))
    bounds.append(n_nodes)

    order = np.argsort(tgt, kind="stable")
    src_s, tgt_s = src[order], tgt[order]
    attr_perm = order  # edge permutation; attr gathered later

    # cut points in the sorted edge array per shard
    eb = [0]
    for i in range(1, NSHARD):
        eb.append(int(np.searchsorted(tgt_s, bounds[i])))
    eb.append(E)

    shards = []
    for s in range(NSHARD):
        sh = Plan()
        sh.n0, sh.n1 = bounds[s], bounds[s + 1]
        sh.e0, sh.e1 = eb[s], eb[s + 1]
        sh.src = src_s[sh.e0:sh.e1]
        sh.tgt = tgt_s[sh.e0:sh.e1]
        sh.attr_eidx = attr_perm[sh.e0:sh.e1]  # original edge ids
        sh.n_own = sh.n1 - sh.n0
        sh.deg = np.bincount(sh.tgt - sh.n0, minlength=sh.n_own)
        # node order: sorted by degree desc (stable)
        sh.node_order = np.argsort(-sh.deg, kind="stable")  # local ids
        sh.sorted_deg = sh.deg[sh.node_order]
        # distinct referenced srcs, ascending; relabel map
        sh.ref = np.unique(sh.src)
        shards.append(sh)

    pl = Plan()
    pl.n_nodes = n_nodes
    pl.E = E
    pl.shards = shards
    pl.S_OWN = _round_up(max(sh.n_own for sh in shards), 512)
    pl.T_TAB = _round_up(max(len(sh.ref) for sh in shards) + 1, 512)
    assert pl.T_TAB - 1 <= 32767, f"table too large for int16 idx: {pl.T_TAB}"
    pl.DUMTOK = pl.T_TAB - 1

    # shared slot schedule: profile[i] = max over shards of sorted_deg[i]
    prof = np.zeros(pl.S_OWN, np.int64)
    for sh in shards:
        prof[: sh.n_own] = np.maximum(prof[: sh.n_own], sh.sorted_deg)
    slot_d = np.array([_bucket(int(d)) for d in prof], np.int64)
    pl.slot_d = slot_d
    pl.n_real_slots = int((slot_d > 0).sum())  # trailing slots are d==0

    # chunking: group consecutive slots, <= CCHUNK padded columns per chunk
    chunks = []
    i = 0
    while i < pl.n_real_slots:
        cols = 0
        j = i
        runs = []  # [col_off, node_off, n_nodes, d]
        while j < pl.n_real_slots:
            d = int(slot_d[j])
            if cols + d > CCHUNK and cols > 0:
                break
            if runs and runs[-1][3] == d:
                runs[-1][2] += 1
            else:
                runs.append([cols, j, 1, d])
            cols += d
            j += 1
        ch = Plan()
        ch.node_off = i
        ch.n_nodes = j - i
        ch.cols = cols
        ch.cpad = _round_up(cols, 128)
        ch.runs = [tuple(r) for r in runs]
        chunks.append(ch)
        i = j
    pl.chunks = chunks
    pl.tot_cols = sum(ch.cpad for ch in chunks)

    # global column offset of every slot (accounting chunk padding)
    col_of_slot = np.zeros(pl.n_real_slots, np.int64)
    off = 0
    for ch in pl.chunks:
        sd = slot_d[ch.node_off: ch.node_off + ch.n_nodes]
        col_of_slot[ch.node_off: ch.node_off + ch.n_nodes] = (
            off + np.concatenate([[0], np.cumsum(sd)[:-1]]))
        off += ch.cpad

    # per-shard slot-expanded idx/attr arrays (vectorized)
    for sh in shards:
        idx = np.full(pl.tot_cols, pl.DUMTOK, np.int64)
        aidx = np.full(pl.tot_cols, -1, np.int64)  # original edge id or -1
        starts = np.zeros(sh.n_own + 1, np.int64)
        np.cumsum(sh.deg, out=starts[1:])
        rel = np.searchsorted(sh.ref, sh.src)  # relabeled src per edge
        npos = min(sh.n_own, pl.n_real_slots)
        nodes = sh.node_order[:npos]
        cnts = sh.deg[nodes]
        tot = int(cnts.sum())
        if tot:
            base = np.concatenate([[0], np.cumsum(cnts)[:-1]])
            within = np.arange(tot) - np.repeat(base, cnts)
            e_src = np.repeat(starts[nodes], cnts) + within
            c_dst = np.repeat(col_of_slot[:npos], cnts) + within
            idx[c_dst] = rel[e_src]
            aidx[c_dst] = sh.attr_eidx[e_src]
        sh.idx_flat = idx
        sh.attr_eid = aidx
    return pl


def build_inputs(pl, x, pre_h, edge_attr, weights):
    """Construct per-core in_maps (list of dicts) for run_bass_kernel_spmd."""
    bf16 = ml_dtypes.bfloat16
    (enc_w, enc_b, M_w, M_b, U_w, U_b, dec_w, dec_b) = weights
    n = pl.n_nodes

    # fold the x column into pre_h when well-conditioned: solve
    # enc_w[1:].T v = enc_w[0]; then enc_w[1:].T (pre_h + x v^T).T
    # reproduces the x contribution exactly and the x matmul disappears.
    v = np.linalg.solve(np.asarray(enc_w[1:1 + H], np.float64).T,
                        np.asarray(enc_w[0], np.float64))
    pl.fold_x = bool(np.abs(v).max() <= 2.0)
    if pl.fold_x:
        pre_h_eff = (np.asarray(pre_h, np.float64) +
                     np.asarray(x, np.float64) @ v[None, :])
    else:
        pre_h_eff = np.asarray(pre_h, np.float64)
    pre_hT = np.ascontiguousarray(pre_h_eff.T).astype(np.float32)  # [H, N]
    pre_hT_bf = pre_hT.astype(bf16)
    x_row = np.ascontiguousarray(x.reshape(1, -1)).astype(np.float32)
    x_row_bf = x_row.astype(bf16)
    attr_flat = np.asarray(edge_attr).reshape(-1)

    # shared weight tensors
    shared = {
        "enc_w_main_bf": enc_w[1:129].astype(bf16),          # [128,128]
        "enc_w_main_f": np.ascontiguousarray(enc_w[1:129], np.float32),
        "enc_w_x_bf": enc_w[0:1].astype(bf16),
        "enc_w_x_f": np.ascontiguousarray(enc_w[0:1], np.float32),
        "m_tgt_bf": M_w[:H].astype(bf16),
        "u_w1_bf": U_w[:H].astype(bf16),
        "u_w2_bf": U_w[H:].astype(bf16),
        "dec_w1_bf": dec_w[:H].astype(bf16),
        "dec_w2_bf": dec_w[H:].astype(bf16),
        "enc_b": np.ascontiguousarray(enc_b.reshape(P, 1), np.float32),
        "m_src_bf": M_w[H:2 * H].astype(bf16),               # [128,128]
        "m_tgt_f": np.ascontiguousarray(M_w[:H], np.float32),
        "m_b_col": np.ascontiguousarray(M_b.reshape(P, 1), np.float32),
        "w_a_col": np.ascontiguousarray(M_w[2 * H].reshape(P, 1), np.float32),
        "u_w1": np.ascontiguousarray(U_w[:H], np.float32),
        "u_w2": np.ascontiguousarray(U_w[H:], np.float32),
        "u_b_col": np.ascontiguousarray(U_b.reshape(P, 1), np.float32),
        "dec_w1": np.ascontiguousarray(dec_w[:H], np.float32),   # [128,1]
        "dec_w2": np.ascontiguousarray(dec_w[H:], np.float32),
        "dec_b_sc": np.ascontiguousarray(
            np.asarray(dec_b).reshape(1, 1), np.float32),
        "neg_row_bf": np.full((1, H), NEG, bf16),
    }

    in_maps = []
    for c in range(NCORES):
        m = dict(shared)
        ph_sh = np.zeros((NSUB, P, pl.T_TAB), bf16)
        x_sh = np.zeros((NSUB, 1, pl.T_TAB), bf16)
        idx_sh = np.zeros((NSUB, P, pl.tot_cols // 16), np.int16)
        attr_sh = np.zeros((NSUB, P, pl.tot_cols), bf16)
        ph_own = np.zeros((NSUB, P, pl.S_OWN), np.float32)
        x_own = np.zeros((NSUB, 1, pl.S_OWN), np.float32)
        for k in range(NSUB):
            sh = pl.shards[c * NSUB + k]
            u = len(sh.ref)
            ph_sh[k, :, :u] = pre_hT_bf[:, sh.ref]
            x_sh[k, :, :u] = x_row_bf[:, sh.ref]
            i16 = sh.idx_flat.astype(np.int16)
            wrapped = i16.reshape(-1, 16).T                  # [16, tot/16]
            idx_sh[k] = np.tile(wrapped, (8, 1))
            av = np.where(sh.attr_eid >= 0,
                          attr_flat[np.maximum(sh.attr_eid, 0)], 0.0)
            attr_sh[k, :] = av.astype(bf16)[None, :]
            own_nodes = sh.node_order + sh.n0                # global ids
            ph_own[k, :, :sh.n_own] = pre_hT[:, own_nodes]
            x_own[k, 0, :sh.n_own] = x_row[0, own_nodes]
        m["pre_h_shard"] = ph_sh
        m["idx_buf"] = idx_sh
        m["attr_buf"] = attr_sh
        m["pre_h_own"] = ph_own
        if not pl.fold_x:
            m["x_shard"] = x_sh
            m["x_own"] = x_own
        in_maps.append(m)
    return in_maps


# ----------------------------------------------------------------------------
# Device kernel
# ----------------------------------------------------------------------------

def build_nc(pl):
    import concourse.bass as bass
    import concourse.bacc as bacc
    import concourse.mybir as mybir
    import concourse.tile as tile

    dt = mybir.dt
    AF = mybir.ActivationFunctionType
    ALU = mybir.AluOpType

    nc = bacc.Bacc("TRN2", target_bir_lowering=False, debug=False,
                   enable_asserts=False, num_devices=NCORES)

    def din(name, shape, dtype):
        return nc.dram_tensor(name, list(shape), dtype,
                              kind="ExternalInput").ap()

    def dout(name, shape, dtype):
        return nc.dram_tensor(name, list(shape), dtype,
                              kind="ExternalOutput").ap()

    T, S = pl.T_TAB, pl.S_OWN
    TC = pl.tot_cols

    pre_h_shard = din("pre_h_shard", (NSUB, P, T), dt.bfloat16)
    fold_x = getattr(pl, "fold_x", True)
    if not fold_x:
        x_shard = din("x_shard", (NSUB, 1, T), dt.bfloat16)
        x_own_t = din("x_own", (NSUB, 1, S), dt.float32)
    idx_buf = din("idx_buf", (NSUB, P, TC // 16), dt.int16)
    attr_buf = din("attr_buf", (NSUB, P, TC), dt.bfloat16)
    pre_h_own = din("pre_h_own", (NSUB, P, S), dt.float32)

    enc_w_main_bf = din("enc_w_main_bf", (P, H), dt.bfloat16)
    enc_w_main_f = din("enc_w_main_f", (P, H), dt.float32)
    enc_w_x_bf = din("enc_w_x_bf", (1, H), dt.bfloat16)
    enc_w_x_f = din("enc_w_x_f", (1, H), dt.float32)
    enc_b = din("enc_b", (P, 1), dt.float32)
    m_tgt_bf = din("m_tgt_bf", (P, H), dt.bfloat16)
    u_w1_bf = din("u_w1_bf", (P, H), dt.bfloat16)
    u_w2_bf = din("u_w2_bf", (P, H), dt.bfloat16)
    dec_w1_bf = din("dec_w1_bf", (P, 1), dt.bfloat16)
    dec_w2_bf = din("dec_w2_bf", (P, 1), dt.bfloat16)
    m_src_bf = din("m_src_bf", (P, H), dt.bfloat16)
    m_tgt_f = din("m_tgt_f", (P, H), dt.float32)
    m_b_col = din("m_b_col", (P, 1), dt.float32)
    w_a_col = din("w_a_col", (P, 1), dt.float32)
    u_w1 = din("u_w1", (P, H), dt.float32)
    u_w2 = din("u_w2", (P, H), dt.float32)
    u_b_col = din("u_b_col", (P, 1), dt.float32)
    dec_w1 = din("dec_w1", (P, 1), dt.float32)
    dec_w2 = din("dec_w2", (P, 1), dt.float32)
    dec_b_sc = din("dec_b_sc", (1, 1), dt.float32)
    neg_row_bf = din("neg_row_bf", (1, H), dt.bfloat16)

    hT_out = dout("hT_out", (NSUB, P, S), dt.float32)
    yT_out = dout("yT_out", (NSUB, 1, S), dt.float32)

    skip = set(os.environ.get("KERNEL_SKIP", "").split(","))
    from contextlib import ExitStack
    with tile.TileContext(nc) as tc, ExitStack() as es:
        wpool = es.enter_context(tc.tile_pool(name="weights", bufs=1))
        dram = es.enter_context(tc.tile_pool(name="dram", bufs=1,
                                             space="DRAM"))
        p1 = es.enter_context(tc.tile_pool(name="p1", bufs=3))
        p1z = es.enter_context(tc.tile_pool(name="p1z", bufs=2))
        p1b = es.enter_context(tc.tile_pool(name="p1b", bufs=3))
        psz = es.enter_context(tc.tile_pool(name="psz", bufs=2, space="PSUM"))
        psb = es.enter_context(tc.tile_pool(name="psb", bufs=2, space="PSUM"))
        p2i = es.enter_context(tc.tile_pool(name="p2i", bufs=2))
        p2a = es.enter_context(tc.tile_pool(name="p2a", bufs=3))
        p2g = es.enter_context(tc.tile_pool(name="p2g", bufs=3))
        p2s = es.enter_context(tc.tile_pool(name="p2s", bufs=2))
        p3 = es.enter_context(tc.tile_pool(name="p3", bufs=2))
        ps3 = es.enter_context(tc.tile_pool(name="ps3", bufs=1, space="PSUM"))
        ps3y = es.enter_context(tc.tile_pool(name="ps3y", bufs=1,
                                             space="PSUM"))

        def wtile(ap, shape, dtype):
            t = wpool.tile(list(shape), dtype, tag=ap.tensor.name,
                           name="w_" + ap.tensor.name)
            nc.sync.dma_start(out=t[:], in_=ap)
            return t

        w_enc_main_bf = wtile(enc_w_main_bf, (P, H), dt.bfloat16)
        w_enc_main_f = wtile(enc_w_main_f, (P, H), dt.float32)
        if not fold_x:
            w_enc_x_bf = wtile(enc_w_x_bf, (1, H), dt.bfloat16)
            w_enc_x_f = wtile(enc_w_x_f, (1, H), dt.float32)
        w_enc_b = wtile(enc_b, (P, 1), dt.float32)
        w_m_tgt_bf = wtile(m_tgt_bf, (P, H), dt.bfloat16)
        w_u1_bf = wtile(u_w1_bf, (P, H), dt.bfloat16)
        w_u2_bf = wtile(u_w2_bf, (P, H), dt.bfloat16)
        w_d1_bf = wtile(dec_w1_bf, (P, 1), dt.bfloat16)
        w_d2_bf = wtile(dec_w2_bf, (P, 1), dt.bfloat16)
        w_m_src = wtile(m_src_bf, (P, H), dt.bfloat16)
        w_m_tgt = wtile(m_tgt_f, (P, H), dt.float32)
        w_m_b = wtile(m_b_col, (P, 1), dt.float32)
        w_wa = wtile(w_a_col, (P, 1), dt.float32)
        w_u1 = wtile(u_w1, (P, H), dt.float32)
        w_u2 = wtile(u_w2, (P, H), dt.float32)
        w_ub = wtile(u_b_col, (P, 1), dt.float32)
        w_d1 = wtile(dec_w1, (P, 1), dt.float32)
        w_d2 = wtile(dec_w2, (P, 1), dt.float32)
        w_db = wtile(dec_b_sc, (1, 1), dt.float32)
        w_neg = wtile(neg_row_bf, (1, H), dt.bfloat16)

        tables = [dram.tile([T, H], dt.bfloat16, tag=f"table{s}",
                            name=f"table{s}") for s in range(NSUB)]
        segmaxs = [p2s.tile([P, S], dt.float32, tag=f"segmax{s}",
                            name=f"segmax{s}") for s in range(NSUB)]
        idxtiles = [None] * NSUB

        # ---------------- emission helpers ----------------
        def emit_p1_group(s, g):
            tab = tables[s]
            ph = p1.tile([P, 512], dt.bfloat16, tag="ph", name="ph")
            nc.sync.dma_start(
                out=ph[:], in_=pre_h_shard[s, :, g * 512:(g + 1) * 512])
            zp = psz.tile([P, 512], dt.float32, space="PSUM", name="zp")
            if fold_x:
                nc.tensor.matmul(zp[:], w_enc_main_bf[:], ph[:],
                                 start=True, stop=True)
            else:
                xx = p1.tile([1, 512], dt.bfloat16, tag="xx", name="xx")
                nc.sync.dma_start(
                    out=xx[:], in_=x_shard[s, :, g * 512:(g + 1) * 512])
                nc.tensor.matmul(zp[:], w_enc_main_bf[:], ph[:],
                                 start=True, stop=False)
                nc.tensor.matmul(zp[:], w_enc_x_bf[:], xx[:],
                                 start=False, stop=True)
            zt = p1z.tile([P, 512], dt.bfloat16, tag="zt", name="zt")
            nc.scalar.activation(zt[:], zp[:], AF.Relu, bias=w_enc_b[:])
            bp = psb.tile([P, 512], dt.float32, space="PSUM", name="bp")
            for q in range(4):
                nc.tensor.matmul(bp[:, q * 128:(q + 1) * 128],
                                 zt[:, q * 128:(q + 1) * 128],
                                 w_m_src[:], start=True, stop=True)
            bs = p1b.tile([P, 512], dt.bfloat16, tag="bs", name="bs")
            nc.scalar.copy(bs[:], bp[:])
            nc.sync.dma_start(
                out=tab[g * 512:(g + 1) * 512, :].rearrange(
                    "(q p) f -> p q f", p=128),
                in_=bs[:].rearrange("p (q f) -> p q f", q=4))

        def emit_p1_tail(s):
            if "p1" not in skip:
                nc.sync.dma_start(out=tables[s][pl.DUMTOK:pl.DUMTOK + 1, :],
                                  in_=w_neg[:])

        def emit_p2_head(s):
            seg = segmaxs[s]
            if "p2" in skip:
                nc.vector.memset(seg[:, :], 0.0)
            if pl.n_real_slots < S:
                nc.vector.memset(seg[:, pl.n_real_slots:], NEG)
            it = p2i.tile([P, TC // 16], dt.int16, tag="idxs", name="idxs")
            nc.scalar.dma_start(out=it[:], in_=idx_buf[s])
            idxtiles[s] = it

        def emit_p2_chunk(s, ch, off):
            seg = segmaxs[s]
            idxs = idxtiles[s]
            at = p2a.tile([P, CCHUNK], dt.bfloat16, tag="attr", name="at")
            nc.scalar.dma_start(out=at[:, :ch.cpad],
                                in_=attr_buf[s, :, off:off + ch.cpad])
            gt = p2g.tile([P, CCHUNK], dt.bfloat16, tag="gath", name="gt")
            if "gather" in skip:
                nc.vector.memset(gt[:, :ch.cpad], 0.0)
            else:
                nc.gpsimd.dma_gather(
                    out_ap=gt[:, :ch.cpad].rearrange("p (o c) -> p o c", o=1),
                    in_ap=tables[s][:, :],
                    idxs_ap=idxs[:, off // 16:(off + ch.cpad) // 16],
                    num_idxs=ch.cpad,
                    num_idxs_reg=ch.cpad,
                    elem_size=H,
                    transpose=True,
                    single_packet=GATHER_SP,
                )
            if "stt" not in skip:
                nc.vector.scalar_tensor_tensor(
                    out=gt[:, :ch.cpad], in0=at[:, :ch.cpad], scalar=w_wa[:],
                    in1=gt[:, :ch.cpad], op0=ALU.mult, op1=ALU.add)
            for (coff, noff, nn, d) in ch.runs:
                nc.vector.reduce_max(
                    out=seg[:, noff:noff + nn],
                    in_=gt[:, coff:coff + nn * d].rearrange(
                        "p (n d) -> p n d", d=d),
                    axis=mybir.AxisListType.X)

        def emit_p3_group(s, g):
            seg = segmaxs[s]
            sl = slice(g * 512, (g + 1) * 512)
            pho = p3.tile([P, 512], dt.float32, tag="pho", name="pho")
            nc.sync.dma_start(out=pho[:], in_=pre_h_own[s, :, sl])
            zp = ps3.tile([P, 512], dt.float32, space="PSUM", tag="zp3",
                          name="zp3")
            if fold_x:
                nc.tensor.matmul(zp[:], w_enc_main_f[:], pho[:],
                                 start=True, stop=True)
            else:
                xo = p3.tile([1, 512], dt.float32, tag="xo", name="xo")
                nc.sync.dma_start(out=xo[:], in_=x_own_t[s, :, sl])
                nc.tensor.matmul(zp[:], w_enc_main_f[:], pho[:],
                                 start=True, stop=False)
                nc.tensor.matmul(zp[:], w_enc_x_f[:], xo[:],
                                 start=False, stop=True)
            zo = p3.tile([P, 512], dt.float32, tag="zo", name="zo")
            nc.scalar.activation(zo[:], zp[:], AF.Relu, bias=w_enc_b[:])

            ap_ = ps3.tile([P, 512], dt.float32, space="PSUM", tag="ap3",
                           name="ap3")
            nc.tensor.matmul(ap_[:], w_m_tgt[:], zo[:], start=True,
                             stop=True)
            u = p3.tile([P, 512], dt.float32, tag="u", name="u")
            nc.vector.tensor_tensor(out=u[:], in0=ap_[:],
                                    in1=seg[:, sl], op=ALU.add)
            agg = p3.tile([P, 512], dt.float32, tag="agg", name="agg")
            nc.scalar.activation(agg[:], u[:], AF.Relu, bias=w_m_b[:])

            hp = ps3.tile([P, 512], dt.float32, space="PSUM", tag="hp3",
                          name="hp3")
            nc.tensor.matmul(hp[:], w_u1[:], zo[:], start=True, stop=False)
            nc.tensor.matmul(hp[:], w_u2[:], agg[:], start=False,
                             stop=True)
            ho = p3.tile([P, 512], dt.float32, tag="ho", name="ho")
            nc.scalar.activation(ho[:], hp[:], AF.Relu, bias=w_ub[:])
            nc.sync.dma_start(out=hT_out[s, :, sl], in_=ho[:])

            yp = ps3y.tile([1, 512], dt.float32, space="PSUM", tag="yp3",
                           name="yp3")
            nc.tensor.matmul(yp[:], w_d1[:], zo[:], start=True, stop=False)
            nc.tensor.matmul(yp[:], w_d2[:], ho[:], start=False,
                             stop=True)
            yo = p3.tile([1, 512], dt.float32, tag="yo", name="yo")
            nc.scalar.activation(yo[:], yp[:], AF.Sigmoid, bias=w_db[:])
            nc.sync.dma_start(out=yT_out[s, :, sl], in_=yo[:])

        # ---------------- emission schedule ----------------
        # p1(0); then p2(s) chunks interleaved with p1(s+1) groups; p3(s)
        # interleaved after.
        ngroups = T // 512 if "p1" not in skip else 0
        for g in range(ngroups):
            emit_p1_group(0, g)
        emit_p1_tail(0)
        for s in range(NSUB):
            emit_p2_head(s)
            nxt = list(range(ngroups)) if s + 1 < NSUB else []
            chunk_list = list(pl.chunks) if "p2" not in skip else []
            nch = max(len(chunk_list), 1)
            per = (len(nxt) + nch - 1) // nch if nxt else 0
            gi = 0
            off = 0
            for ci, ch in enumerate(chunk_list):
                emit_p2_chunk(s, ch, off)
                off += ch.cpad
                for _ in range(per):
                    if gi < len(nxt):
                        emit_p1_group(s + 1, nxt[gi])
                        gi += 1
            while gi < len(nxt):
                emit_p1_group(s + 1, nxt[gi])
                gi += 1
            if s + 1 < NSUB:
                emit_p1_tail(s + 1)
            for g in range(S // 512):
                emit_p3_group(s, g)

    nc.compile()
    return nc


# ----------------------------------------------------------------------------
# Public entry point
# ----------------------------------------------------------------------------

def _install_ntff_hook():
    """Make trace=True work under axon in images whose antenv lacks
    axon_hooks (degrades to no-trace on any failure)."""
    try:
        import types
        import antenv
        if "antenv.axon_hooks" not in sys.modules:
            mod = types.ModuleType("antenv.axon_hooks")
            _h = [None]
            mod.set_axon_ntff_profile_hook = lambda h: _h.__setitem__(0, h)
            mod.get_axon_ntff_profile_hook = lambda: _h[0]
            sys.modules["antenv.axon_hooks"] = mod
            antenv.axon_hooks = mod
        from antenv import axon_hooks
        if axon_hooks.get_axon_ntff_profile_hook() is None:
            from trn_agent_boot.trn_boot import _ntff_profile_via_ctypes
            axon_hooks.set_axon_ntff_profile_hook(
                _ntff_profile_via_ctypes("/opt/axon/libaxon_pjrt.so"))
    except Exception as e:  # noqa: BLE001
        print(f"ntff hook install failed ({e}); tracing disabled")


def kernel(x, pre_h, edge_attr, enc_w, enc_b, M_w, M_b, U_w, U_b,
           dec_w, dec_b, ter_w, ter_b, edge_index):
    from concourse import bass_utils
    trace = bool(int(os.environ.get("KERNEL_TRACE", "0")))
    if trace:
        _install_ntff_hook()

    x = np.asarray(x, np.float32)
    pre_h = np.asarray(pre_h, np.float32)
    edge_attr = np.asarray(edge_attr, np.float32)
    edge_index = np.asarray(edge_index)
    n_nodes = x.shape[0]

    pl = build_plan(edge_index, n_nodes)
    in_maps = build_inputs(pl, x, pre_h, edge_attr,
                           (np.asarray(enc_w, np.float32),
                            np.asarray(enc_b, np.float32),
                            np.asarray(M_w, np.float32),
                            np.asarray(M_b, np.float32),
                            np.asarray(U_w, np.float32),
                            np.asarray(U_b, np.float32),
                            np.asarray(dec_w, np.float32),
                            np.asarray(dec_b, np.float32)))
    nc = build_nc(pl)
    res = bass_utils.run_bass_kernel_spmd(
        nc, in_maps, core_ids=list(range(NCORES)), trace=trace)

    h = np.zeros((n_nodes, H), np.float32)
    y = np.zeros((n_nodes, 1), np.float32)
    for c in range(NCORES):
        out = res.results[c]
        for k in range(NSUB):
            sh = pl.shards[c * NSUB + k]
            nodes = sh.node_order + sh.n0
            h[nodes] = out["hT_out"][k, :, :sh.n_own].T
            y[nodes, 0] = out["yT_out"][k, 0, :sh.n_own]

    ter_w = np.asarray(ter_w, np.float32)
    hm = h.mean(0)
    ter = np.float32((hm @ (ter_w[:H, 0] + ter_w[H:, 0])) +
                     np.asarray(ter_b, np.float32).reshape(-1)[0])
    kernel.last_results = res
    return h, y, ter
